# revision 1
# baseline (speedup 1.0000x reference)
"""Bass/Trainium2 kernel for nn_DimeNet_22737556865501.

Strategy (v2)
-------------
Same circulant-structure collapse as v1 (per-atom dense math on the 16
local displacement vectors), but restructured around three observations:

1. Normalize first: with Vhat = V/|V|, the Gram matrix of Vhat IS cos(alpha),
   eliminating the ab/amg/den/ln/exp chain entirely.
2. Half-angle: alpha = 2*atan(sqrt((1-c)/(1+c))) and
   (1-c)/(1+c) = 2/(1+c) - 1, so alpha costs exactly three ACT ops
   (reciprocal with bias, sqrt with scale/bias, arctan) after one clamp.
   The factor 2 folds into env2 = 2*env (already needed).
3. bf16 + DVE 2x packing for the two dense blocks (Gram pair products and
   the alpha@erbf contraction), with the c-dimension padded to 4 so the
   innermost axis stays even/step-1; reductions use DVE tensor_reduce or
   short trees with f32 final accumulation.

Activation tables used (one load each, phases globally ordered):
  reciprocal_sqrt_and_small (Square, Rsqrt) -> reciprocal_and_small
  (Reciprocal) -> sqrt_and_others (Sqrt) -> trig_and_small (Sin, Arctan).

Sharding: atoms partitioned across the 8 NeuronCores (4096 each); each core
writes its own 65536x6 output rows; host concatenates. Host verifies the
circulant graph and falls back to exact numpy otherwise.
"""

import numpy as np

N_ATOMS = 32768
DEG = 16
HALF = DEG // 2
N_CORES = 8
J_PER_CORE = N_ATOMS // N_CORES  # 4096
P = 128  # partitions / atoms per tile
N_TILES = J_PER_CORE // P  # 32
WIN_ROWS = J_PER_CORE + DEG  # 4112 (8-row halo each side)
N_RBF = 6
CUTOFF = 5.0
ENV_P = 6
A_ = -(ENV_P + 1) * (ENV_P + 2) / 2.0  # -28
B_ = float(ENV_P * (ENV_P + 2))  # 48
C_ = -ENV_P * (ENV_P + 1) / 2.0  # -21
EA = 2.0 * A_  # -56
EB = 2.0 * B_  # 96
EC = 2.0 * C_  # -42
TWO_PI = float(2.0 * np.pi)

# tile ownership: t % 8 < GD_DVE -> Gram on DVE; t % 8 < CD_DVE -> contraction
# on DVE (rest on Pool/GpSimd)
GD_DVE = 5
CD_DVE = 7

OFFS = np.concatenate([np.arange(1, HALF + 1), -np.arange(1, HALF + 1)])

_cached_nc = None


def _expected_graph():
    half = HALF
    offs = np.concatenate([np.arange(1, half + 1), N_ATOMS - np.arange(1, half + 1)])
    j = np.arange(N_ATOMS)
    nbr_dst = (j[:, None] + offs[None, :]) % N_ATOMS
    nbr_list = np.stack([np.repeat(j, DEG), nbr_dst.reshape(-1)], 1)
    o1, o2 = np.meshgrid(offs, offs, indexing="ij")
    keep = o1 != o2
    o1, o2 = o1[keep], o2[keep]
    i = (j[:, None] + o1[None, :]) % N_ATOMS
    k = (j[:, None] + o2[None, :]) % N_ATOMS
    jc = np.broadcast_to(j[:, None], i.shape)
    angle_list = np.stack([i.reshape(-1), jc.reshape(-1), k.reshape(-1)], 1)
    return nbr_list.astype(np.int64), angle_list.astype(np.int64)


def _graph_matches(nbr_list, angle_list):
    if nbr_list.shape != (N_ATOMS * DEG, 2):
        return False
    if angle_list.shape != (N_ATOMS * DEG * (DEG - 1), 3):
        return False
    exp_nbr, exp_ang = _expected_graph()
    return np.array_equal(np.asarray(nbr_list), exp_nbr) and np.array_equal(
        np.asarray(angle_list), exp_ang
    )


def _fallback_numpy(xyz, nbr_list, angle_list):
    """Exact numpy mirror of the jax reference (general graph)."""
    xyz = np.asarray(xyz, dtype=np.float32)
    nbr = np.asarray(nbr_list)
    ang = np.asarray(angle_list)
    E = nbr.shape[0]
    r_ji = xyz[ang[:, 0]] - xyz[ang[:, 1]]
    r_jk = xyz[ang[:, 2]] - xyz[ang[:, 1]]
    dot = np.sum(r_ji * r_jk, axis=-1)
    crs = np.linalg.norm(np.cross(r_ji, r_jk), axis=-1)
    alpha = np.arctan2(crs, dot)
    diff = xyz[nbr[:, 0]] - xyz[nbr[:, 1]]
    d = np.linalg.norm(diff, axis=-1)
    n = np.arange(1, N_RBF + 1, dtype=xyz.dtype)
    dc = (d / CUTOFF)[:, None]
    env = 1.0 / dc + A_ * dc ** (ENV_P - 1) + B_ * dc**ENV_P + C_ * dc ** (ENV_P + 1)
    e_rbf = env * np.sin(n * np.pi * dc)
    keys = nbr[:, 0] * N_ATOMS + nbr[:, 1]
    order = np.argsort(keys, kind="stable")
    ji_idx = order[np.searchsorted(keys[order], ang[:, 1] * N_ATOMS + ang[:, 0])]
    kj_idx = order[np.searchsorted(keys[order], ang[:, 2] * N_ATOMS + ang[:, 1])]
    trip = alpha[:, None] * e_rbf[kj_idx]
    out = np.zeros((E, N_RBF), dtype=np.float32)
    np.add.at(out, ji_idx, trip.astype(np.float32))
    return out


# ---------------------------------------------------------------------------
# Device kernel
# ---------------------------------------------------------------------------


def _build_device_kernel():
    import concourse.bacc as bacc
    import concourse.mybir as mybir
    from concourse.bass_types import AP
    from concourse.tile import TileContext

    F32 = mybir.dt.float32
    BF16 = mybir.dt.bfloat16
    I32 = mybir.dt.int32
    ALU = mybir.AluOpType
    ACT = mybir.ActivationFunctionType
    AX = mybir.AxisListType

    # Steer the activation-table-load pass so each function resolves to one
    # set and the phase ordering needs exactly four table loads.
    from concourse.hw_specs import get_activation_tables

    assign = {
        ACT.Square: "natural_log_exp_and_others",
        ACT.Ln: "natural_log_exp_and_others",
        ACT.Exp: "natural_log_exp_and_others",
        ACT.Sqrt: "sqrt_and_others",
        ACT.Sin: "trig_and_small",
        ACT.Arctan: "trig_and_small",
    }
    tabs = get_activation_tables("gen3")
    for name, fns in tabs.items():
        for fn, keep in assign.items():
            if name != keep:
                fns.discard(fn)

    def sub(base: AP, off: int, dims) -> AP:
        """Sub-AP of an SBUF tile: keep partition dim, custom free dims."""
        return AP(
            tensor=base.tensor,
            offset=base.offset + off,
            ap=[list(base.ap[0]), *[list(d) for d in dims]],
        )

    T = N_TILES  # 32
    CH = 4  # pipeline chunks
    TC = T // CH  # tiles per chunk
    nc = bacc.Bacc("TRN2", target_bir_lowering=False, debug=False, num_devices=N_CORES)
    win = nc.dram_tensor("win", [WIN_ROWS, 3], F32, kind="ExternalInput")
    consts = nc.dram_tensor("consts", [P, 16], F32, kind="ExternalInput")
    out = nc.dram_tensor("out", [J_PER_CORE * DEG, N_RBF], F32, kind="ExternalOutput")

    with TileContext(nc) as tc:
        with (
            tc.tile_pool(name="big", bufs=1) as big,
            tc.tile_pool(name="work", bufs=2) as work,
        ):
            nco = big.tile([P, 16], F32, name="nco")
            nc.sync.dma_start(nco[:], consts[:])
            # register -1.0 (held in consts slot 7) for activation bias use
            nc.const_aps.aps[(F32, -1.0)] = sub(nco[:], 7, [[1, 1]])

            # ---- global buffers (per-partition free sizes) ----
            w = big.tile([P, T * 51], F32, name="w")  # window
            v = big.tile([P, T * 48], F32, name="v")  # V f32 [t,b,3]
            n2 = big.tile([P, T * 16], F32, name="n2")
            yn = big.tile([P, T * 16], F32, name="yn")  # 1/d
            dd = big.tile([P, T * 16], F32, name="dd")  # d
            vh = big.tile([P, T * 48], F32, name="vh")  # Vhat f32 [t,b,3]
            dc = big.tile([P, T * 16], F32, name="dc")
            q = big.tile([P, T * 16], F32, name="q")
            x5 = big.tile([P, T * 16], F32, name="x5")
            h1 = big.tile([P, T * 16], F32, name="h1")
            env2 = big.tile([P, T * 16], F32, name="env2")
            sa2 = big.tile([P, T * 96], F32, name="sa2")  # [t,r,b] turns
            ki = big.tile([P, T * 96], I32, name="ki")
            kf = big.tile([P, T * 96], F32, name="kf")
            sinv = big.tile([P, T * 96], F32, name="sinv")
            erbf = big.tile([P, T * 96], BF16, name="erbf")  # [t,r,b]
            ch = big.tile([P, T * 256], F32, name="ch")  # cos alpha [t,b,a]
            rec = big.tile([P, T * 256], F32, name="rec")
            al2 = big.tile([P, T * 256], BF16, name="al2")  # alpha/2 bf16
            ot = big.tile([P, T * 96], F32, name="ot")  # out [t,a,r]

            def R(buf, c, per, dims):
                return sub(buf[:], c * TC * per, dims)

            # ---------- phase-major, chunk-minor pipeline ----------
            # window loads (one DMA per chunk)
            for c in range(CH):
                src = AP(
                    tensor=win,
                    offset=c * TC * P * 3,
                    ap=[[3, P], [P * 3, TC], [1, 51]],
                )
                nc.sync.dma_start(R(w, c, 51, [[1, TC * 51]]), src)

            # V[t,a,c]; a=0..7 <- +1..+8, a=8..15 <- -1..-8  (Pool)
            for c in range(CH):
                ctr = R(w, c, 51, [[51, TC], [0, 8], [1, 3]])
                ctr = AP(tensor=ctr.tensor, offset=ctr.offset + 24, ap=ctr.ap)
                nc.gpsimd.tensor_tensor(
                    R(v, c, 48, [[48, TC], [3, 8], [1, 3]]),
                    sub(R(w, c, 51, [[51, TC], [3, 8], [1, 3]]), 27, None)
                    if False
                    else AP(
                        tensor=w.tensor,
                        offset=w[:].offset + c * TC * 51 + 27,
                        ap=[list(w[:].ap[0]), [51, TC], [3, 8], [1, 3]],
                    ),
                    ctr,
                    ALU.subtract,
                )
                nc.gpsimd.tensor_tensor(
                    AP(
                        tensor=v.tensor,
                        offset=v[:].offset + c * TC * 48 + 24,
                        ap=[list(v[:].ap[0]), [48, TC], [3, 8], [1, 3]],
                    ),
                    AP(
                        tensor=w.tensor,
                        offset=w[:].offset + c * TC * 51 + 21,
                        ap=[list(w[:].ap[0]), [51, TC], [-3, 8], [1, 3]],
                    ),
                    ctr,
                    ALU.subtract,
                )

            # ---- norms: n2 = sum_c V^2 (Pool, 5 ops using q as scratch) ----
            for c in range(CH):
                vv0 = R(v, c, 48, [[48, TC], [3, 16], [0, 1]])
                vv1 = AP(tensor=v.tensor, offset=v[:].offset + c * TC * 48 + 1,
                         ap=[list(v[:].ap[0]), [48, TC], [3, 16], [0, 1]])
                vv2 = AP(tensor=v.tensor, offset=v[:].offset + c * TC * 48 + 2,
                         ap=[list(v[:].ap[0]), [48, TC], [3, 16], [0, 1]])
                n2c = R(n2, c, 16, [[16, TC], [1, 16], [0, 1]])
                qc = R(q, c, 16, [[16, TC], [1, 16], [0, 1]])
                nc.gpsimd.tensor_tensor(n2c, vv0, vv0, ALU.mult)
                nc.gpsimd.tensor_tensor(qc, vv1, vv1, ALU.mult)
                nc.gpsimd.tensor_tensor(n2c, n2c, qc, ALU.add)
                nc.gpsimd.tensor_tensor(qc, vv2, vv2, ALU.mult)
                nc.gpsimd.tensor_tensor(n2c, n2c, qc, ALU.add)

            # ---- 1/d via exp(-0.5 ln n2) (ACT, natural_log_exp set) ----
            for c in range(CH):
                nc.scalar.activation(
                    R(yn, c, 16, [[1, TC * 16]]), R(n2, c, 16, [[1, TC * 16]]),
                    ACT.Ln,
                )
                nc.scalar.activation(
                    R(yn, c, 16, [[1, TC * 16]]), R(yn, c, 16, [[1, TC * 16]]),
                    ACT.Exp, scale=-0.5,
                )

            for c in range(CH):
                # d = n2 * yn ; Vhat = V * yn (DVE)
                nc.vector.tensor_tensor(
                    R(dd, c, 16, [[1, TC * 16]]),
                    R(n2, c, 16, [[1, TC * 16]]),
                    R(yn, c, 16, [[1, TC * 16]]),
                    ALU.mult,
                )
                nc.vector.tensor_tensor(
                    R(vh, c, 48, [[48, TC], [3, 16], [1, 3]]),
                    R(v, c, 48, [[48, TC], [3, 16], [1, 3]]),
                    R(yn, c, 16, [[16, TC], [1, 16], [0, 3]]),
                    ALU.mult,
                )

            # ---- envelope: env2 = 2C*yn + dc^5*(EA + EB*dc + EC*dc^2) ----
            for c in range(CH):
                dcc = R(dc, c, 16, [[1, TC * 16]])
                qc = R(q, c, 16, [[1, TC * 16]])
                x5c = R(x5, c, 16, [[1, TC * 16]])
                h1c = R(h1, c, 16, [[1, TC * 16]])
                nc.vector.tensor_scalar(
                    dcc, R(dd, c, 16, [[1, TC * 16]]), 1.0 / CUTOFF, None, ALU.mult
                )
                nc.gpsimd.tensor_tensor(qc, dcc, dcc, ALU.mult)
                nc.gpsimd.tensor_tensor(x5c, qc, qc, ALU.mult)
                nc.gpsimd.tensor_tensor(x5c, x5c, dcc, ALU.mult)
                nc.vector.tensor_scalar(h1c, dcc, EC, EB, ALU.mult, ALU.add)
                nc.gpsimd.tensor_tensor(h1c, h1c, dcc, ALU.mult)
                nc.vector.tensor_scalar(h1c, h1c, EA, None, ALU.add)
                nc.gpsimd.tensor_tensor(x5c, x5c, h1c, ALU.mult)
                nc.vector.scalar_tensor_tensor(
                    R(env2, c, 16, [[1, TC * 16]]),
                    R(yn, c, 16, [[1, TC * 16]]),
                    2.0 * CUTOFF, x5c, ALU.mult, ALU.add,
                )

            # ---- sin args (turns) + range reduction ----
            for c in range(CH):
                nc.vector.tensor_tensor(
                    R(sa2, c, 96, [[96, TC], [16, 6], [1, 16]]),
                    R(dd, c, 16, [[16, TC], [0, 6], [1, 16]]),
                    sub(nco[:], 0, [[0, TC], [1, 6], [0, 16]]),
                    ALU.mult,
                )
                nc.vector.tensor_copy(
                    R(ki, c, 96, [[1, TC * 96]]), R(sa2, c, 96, [[1, TC * 96]])
                )
                nc.vector.tensor_copy(
                    R(kf, c, 96, [[1, TC * 96]]), R(ki, c, 96, [[1, TC * 96]])
                )
                nc.gpsimd.tensor_tensor(
                    R(sa2, c, 96, [[1, TC * 96]]),
                    R(sa2, c, 96, [[1, TC * 96]]),
                    R(kf, c, 96, [[1, TC * 96]]),
                    ALU.subtract,
                )

            # ---- Gram (Pool, f32): ch[t,b,a] = sum_c Vhat[b,c]*Vhat[a,c] ----
            for c in range(CH):
                for tl in range(TC):
                    t = c * TC + tl
                    p3 = work.tile([P, 768], F32, tag="p3", bufs=2)
                    nc.gpsimd.tensor_tensor(
                        sub(p3[:], 0, [[48, 16], [3, 16], [1, 3]]),
                        sub(vh[:], t * 48, [[0, 16], [3, 16], [1, 3]]),
                        sub(vh[:], t * 48, [[3, 16], [0, 16], [1, 3]]),
                        ALU.mult,
                    )
                    nc.gpsimd.tensor_tensor(
                        sub(ch[:], t * 256, [[1, 256]]),
                        sub(p3[:], 0, [[3, 256]]),
                        sub(p3[:], 1, [[3, 256]]),
                        ALU.add,
                    )
                    nc.gpsimd.tensor_tensor(
                        sub(ch[:], t * 256, [[1, 256]]),
                        sub(ch[:], t * 256, [[1, 256]]),
                        sub(p3[:], 2, [[3, 256]]),
                        ALU.add,
                    )

            # clamp so both ln args stay positive: |c| <= 1 - 2^-23
            for c in range(CH):
                nc.vector.tensor_scalar(
                    R(ch, c, 256, [[1, TC * 256]]),
                    R(ch, c, 256, [[1, TC * 256]]),
                    0.9999999, -0.9999999, ALU.min, ALU.max,
                )

            # ---- alpha/2 = atan(exp(0.5*(ln(1-c) - ln(1+c)))) ----
            for c in range(CH):
                cc = R(ch, c, 256, [[1, TC * 256]])
                rc = R(rec, c, 256, [[1, TC * 256]])
                nc.scalar.activation(rc, cc, ACT.Ln, bias=1.0)  # ln(1+c)
                nc.scalar.activation(cc, cc, ACT.Ln, bias=1.0, scale=-1.0)
                nc.vector.tensor_tensor(cc, cc, rc, ALU.subtract)
                nc.scalar.activation(rc, cc, ACT.Exp, scale=0.5)
            # sin goes through the same-set boundary; trig set loads once
            for c in range(CH):
                nc.scalar.activation(
                    R(al2, c, 256, [[1, TC * 256]]),
                    R(rec, c, 256, [[1, TC * 256]]),
                    ACT.Arctan,
                )
                nc.scalar.activation(
                    R(sinv, c, 96, [[1, TC * 96]]),
                    R(sa2, c, 96, [[1, TC * 96]]),
                    ACT.Sin, scale=TWO_PI,
                )
            # diagonal alpha := 0 (reference's i != k exclusion)
            for c in range(CH):
                dg = AP(
                    tensor=al2.tensor,
                    offset=al2[:].offset + c * TC * 256,
                    ap=[list(al2[:].ap[0]), [256, TC], [17, 16]],
                )
                nc.vector.tensor_scalar_mul(dg, dg, 0.0)

            # ---- e_rbf (x2): erbf[t,r,b] = env2[t,b] * sinv[t,r,b] (Pool) ----
            for c in range(CH):
                nc.gpsimd.tensor_tensor(
                    R(erbf, c, 96, [[96, TC], [16, 6], [1, 16]]),
                    R(sinv, c, 96, [[96, TC], [16, 6], [1, 16]]),
                    R(env2, c, 16, [[16, TC], [0, 6], [1, 16]]),
                    ALU.mult,
                )

            # ---- contraction (DVE): ot[t,a,r] = sum_b al2[t,a,b]*erbf[t,r,b]
            for c in range(CH):
                for tl in range(TC):
                    t = c * TC + tl
                    p4 = work.tile([P, 1536], BF16, tag="p4", bufs=2)
                    t1 = work.tile([P, 768], BF16, tag="t1", bufs=2)
                    t2 = work.tile([P, 384], BF16, tag="t2", bufs=2)
                    t3 = work.tile([P, 192], F32, tag="t3", bufs=2)
                    nc.vector.tensor_tensor(
                        sub(p4[:], 0, [[96, 16], [16, 6], [1, 16]]),
                        sub(al2[:], t * 256, [[16, 16], [0, 6], [1, 16]]),
                        sub(erbf[:], t * 96, [[0, 16], [16, 6], [1, 16]]),
                        ALU.mult,
                    )
                    nc.vector.tensor_tensor(
                        sub(t1[:], 0, [[8, 96], [1, 8]]),
                        sub(p4[:], 0, [[16, 96], [1, 8]]),
                        sub(p4[:], 8, [[16, 96], [1, 8]]),
                        ALU.add,
                    )
                    nc.vector.tensor_tensor(
                        sub(t2[:], 0, [[4, 96], [1, 4]]),
                        sub(t1[:], 0, [[8, 96], [1, 4]]),
                        sub(t1[:], 4, [[8, 96], [1, 4]]),
                        ALU.add,
                    )
                    nc.vector.tensor_tensor(
                        sub(t3[:], 0, [[2, 96], [1, 2]]),
                        sub(t2[:], 0, [[4, 96], [1, 2]]),
                        sub(t2[:], 2, [[4, 96], [1, 2]]),
                        ALU.add,
                    )
                    nc.vector.tensor_tensor(
                        sub(ot[:], t * 96, [[1, 96]]),
                        sub(t3[:], 0, [[2, 96]]),
                        sub(t3[:], 1, [[2, 96]]),
                        ALU.add,
                    )
                # out rows (t*128 + p)*16 + a, cols r  (one DMA per chunk)
                dst = AP(
                    tensor=out,
                    offset=c * TC * P * 96,
                    ap=[[96, P], [96 * P, TC], [1, 96]],
                )
                nc.sync.dma_start(dst, R(ot, c, 96, [[96, TC], [1, 96]]))

    nc.compile()
    return nc


def _get_nc():
    global _cached_nc
    if _cached_nc is None:
        _cached_nc = _build_device_kernel()
    return _cached_nc


def _make_consts():
    ncv = np.zeros(16, np.float32)
    ncv[:6] = (np.arange(1, N_RBF + 1) / (2.0 * CUTOFF)).astype(np.float32)
    ncv[7] = -1.0
    return np.broadcast_to(ncv, (P, 16)).copy()


def _run_device(xyz, trace=False, tmpdir=None):
    from concourse import bass_utils

    nc = _get_nc()
    consts = _make_consts()
    ext = np.concatenate([xyz[-HALF:], xyz, xyz[:HALF]], axis=0)  # halo-extended
    in_maps = []
    for c in range(N_CORES):
        base = c * J_PER_CORE
        winc = np.ascontiguousarray(ext[base : base + WIN_ROWS])
        in_maps.append({"win": winc, "consts": consts})
    kwargs = {}
    if trace:
        kwargs = dict(trace=True)
        if tmpdir is not None:
            kwargs["tmpdir"] = tmpdir
    res = bass_utils.run_bass_kernel_spmd(
        nc, in_maps, core_ids=list(range(N_CORES)), **kwargs
    )
    shards = [res.results[c]["out"] for c in range(N_CORES)]
    full = np.concatenate(shards, axis=0).astype(np.float32)
    return full, res


def kernel(xyz, nbr_list, angle_list):
    xyz = np.asarray(xyz, dtype=np.float32)
    if not _graph_matches(nbr_list, angle_list):
        return _fallback_numpy(xyz, nbr_list, angle_list)
    out, _ = _run_device(xyz)
    return out



# revision 3
# speedup vs baseline: 1.2398x; 1.2398x over previous
"""Bass/Trainium2 kernel for nn_DimeNet_22737556865501.

Strategy (v3)
-------------
Circulant-structure collapse (per-atom dense math on the 16 local
displacement vectors), restructured from v2 around trace findings:

- GpSimd (Pool) runs ~2.2-3 ns/elem vs DVE 0.52 (f16 2x) / 1.04 (f32):
  shift the bulk to DVE in fp16, keep Pool on V-build/envelope/partial
  Gram (f32, precision anchor).
- fp16 (not bf16) for alpha/erbf/contraction: 8x lower rounding error,
  2x DVE mode confirmed (incl. outer-broadcast APs). erbf carries a
  1/64 scale to stay in f16 range; the final f32 stage multiplies by 64.
- e_rbf sin harmonics via the scale-invariant Chebyshev recurrence
  E_n = 2cos(theta) E_{n-1} - E_{n-2} seeded from two in-range Sin
  calls (args in [0, 2pi)); kills the per-harmonic range-reduction
  casts of v2.
- alpha chain: clamp, Ln((1-c)/2), Ln((1+c)/2), f16 subtract,
  Exp(0.5 dln) = tan(alpha/2), Arctan -> alpha/2 in f16. Two ACT
  tables total (natural_log_exp + trig).
- Gram cos matrix split by tile: K_POOL tiles/chunk on Pool (f32,
  3 chunk-wide product instrs, one per coordinate) and the rest on DVE
  (f16, per-tile products over a c4-padded layout, 2x mode).

Sharding: atoms partitioned across the 8 NeuronCores (4096 each); each
core writes its own 65536x6 output rows; host concatenates. Host
verifies the circulant graph and falls back to exact numpy otherwise.
"""

import numpy as np

N_ATOMS = 32768
DEG = 16
HALF = DEG // 2
N_CORES = 8
J_PER_CORE = N_ATOMS // N_CORES  # 4096
P = 128  # partitions / atoms per tile
T = J_PER_CORE // P  # 32 tiles
CH = 4  # chunks (gram/alpha/contraction granularity)
TC = T // CH  # 8 tiles per chunk
SC = 2  # superchunks (prep granularity)
SCT = T // SC  # 16 tiles per superchunk
WIN_ROWS = J_PER_CORE + DEG  # 4112 (8-row halo each side)
N_RBF = 6
CUTOFF = 5.0
ENV_P = 6
A_ = -(ENV_P + 1) * (ENV_P + 2) / 2.0  # -28
B_ = float(ENV_P * (ENV_P + 2))  # 48
C_ = -ENV_P * (ENV_P + 1) / 2.0  # -21
OSCALE = 64.0  # erbf carries 1/OSCALE; final f32 stage multiplies back
EA64 = 2.0 * A_ / OSCALE
EB64 = 2.0 * B_ / OSCALE
EC64 = 2.0 * C_ / OSCALE
YN64 = 2.0 * CUTOFF / OSCALE  # (2/64) * (1/dc) = YN64 * yn
PI = float(np.pi)

K_POOL = 6  # gram tiles per chunk-of-8 computed on Pool (f32); rest DVE f16

_cached_nc = None


def _expected_graph():
    half = HALF
    offs = np.concatenate([np.arange(1, half + 1), N_ATOMS - np.arange(1, half + 1)])
    j = np.arange(N_ATOMS)
    nbr_dst = (j[:, None] + offs[None, :]) % N_ATOMS
    nbr_list = np.stack([np.repeat(j, DEG), nbr_dst.reshape(-1)], 1)
    o1, o2 = np.meshgrid(offs, offs, indexing="ij")
    keep = o1 != o2
    o1, o2 = o1[keep], o2[keep]
    i = (j[:, None] + o1[None, :]) % N_ATOMS
    k = (j[:, None] + o2[None, :]) % N_ATOMS
    jc = np.broadcast_to(j[:, None], i.shape)
    angle_list = np.stack([i.reshape(-1), jc.reshape(-1), k.reshape(-1)], 1)
    return nbr_list.astype(np.int64), angle_list.astype(np.int64)


def _graph_matches(nbr_list, angle_list):
    if nbr_list.shape != (N_ATOMS * DEG, 2):
        return False
    if angle_list.shape != (N_ATOMS * DEG * (DEG - 1), 3):
        return False
    exp_nbr, exp_ang = _expected_graph()
    return np.array_equal(np.asarray(nbr_list), exp_nbr) and np.array_equal(
        np.asarray(angle_list), exp_ang
    )


def _fallback_numpy(xyz, nbr_list, angle_list):
    """Exact numpy mirror of the jax reference (general graph)."""
    xyz = np.asarray(xyz, dtype=np.float32)
    nbr = np.asarray(nbr_list)
    ang = np.asarray(angle_list)
    E = nbr.shape[0]
    r_ji = xyz[ang[:, 0]] - xyz[ang[:, 1]]
    r_jk = xyz[ang[:, 2]] - xyz[ang[:, 1]]
    dot = np.sum(r_ji * r_jk, axis=-1)
    crs = np.linalg.norm(np.cross(r_ji, r_jk), axis=-1)
    alpha = np.arctan2(crs, dot)
    diff = xyz[nbr[:, 0]] - xyz[nbr[:, 1]]
    d = np.linalg.norm(diff, axis=-1)
    n = np.arange(1, N_RBF + 1, dtype=xyz.dtype)
    dc = (d / CUTOFF)[:, None]
    env = 1.0 / dc + A_ * dc ** (ENV_P - 1) + B_ * dc**ENV_P + C_ * dc ** (ENV_P + 1)
    e_rbf = env * np.sin(n * np.pi * dc)
    keys = nbr[:, 0] * N_ATOMS + nbr[:, 1]
    order = np.argsort(keys, kind="stable")
    ji_idx = order[np.searchsorted(keys[order], ang[:, 1] * N_ATOMS + ang[:, 0])]
    kj_idx = order[np.searchsorted(keys[order], ang[:, 2] * N_ATOMS + ang[:, 1])]
    trip = alpha[:, None] * e_rbf[kj_idx]
    out = np.zeros((E, N_RBF), dtype=np.float32)
    np.add.at(out, ji_idx, trip.astype(np.float32))
    return out


# ---------------------------------------------------------------------------
# Device kernel
# ---------------------------------------------------------------------------


def _build_device_kernel():
    import concourse.bacc as bacc
    import concourse.mybir as mybir
    from concourse.bass_types import AP
    from concourse.tile import TileContext

    F32 = mybir.dt.float32
    F16 = mybir.dt.float16
    I32 = mybir.dt.int32
    ALU = mybir.AluOpType
    ACT = mybir.ActivationFunctionType

    KP = K_POOL
    KD = TC - KP  # DVE gram tiles per chunk

    nc = bacc.Bacc("TRN2", target_bir_lowering=False, debug=False, num_devices=N_CORES)
    win = nc.dram_tensor("win", [WIN_ROWS, 3], F32, kind="ExternalInput")
    consts = nc.dram_tensor("consts", [P, 8], F32, kind="ExternalInput")
    out = nc.dram_tensor("out", [J_PER_CORE * DEG, N_RBF], F32, kind="ExternalOutput")

    def sub(base: AP, off: int, dims) -> AP:
        """Sub-AP of an SBUF tile: keep partition dim, custom free dims."""
        return AP(
            tensor=base.tensor,
            offset=base.offset + off,
            ap=[list(base.ap[0]), *[list(d) for d in dims]],
        )

    with TileContext(nc) as tc:
        with (
            tc.tile_pool(name="big", bufs=1) as big,
            tc.tile_pool(name="work", bufs=2) as work,
        ):
            nco = big.tile([P, 8], F32, name="nco")
            nc.sync.dma_start(nco[:], consts[:])
            # register 0.5 (slot 0) for activation bias use
            nc.const_aps.aps[(F32, 0.5)] = sub(nco[:], 0, [[1, 1]])

            # ---- persistent per-core buffers (free sizes per partition) ----
            w = big.tile([P, T * 51], F32, name="w")
            vc = big.tile([P, T * 48], F32, name="vc")  # V [t,c,b] f32
            n2a = big.tile([P, T * 16], F32, name="n2a")
            n2 = big.tile([P, T * 16], F32, name="n2")
            yn = big.tile([P, T * 16], F32, name="yn")  # 1/d
            xh = big.tile([P, T * 16], F32, name="xh")  # d/(2C)
            dcb = big.tile([P, T * 16], F32, name="dcb")  # d/C
            vh32 = big.tile([P, T * 48], F32, name="vh32")  # Vhat [t,c,b] f32
            vh16 = big.tile([P, T * 64], F16, name="vh16")  # Vhat [t,b,c4] f16
            qq = big.tile([P, T * 16], F32, name="qq")
            x4 = big.tile([P, T * 16], F32, name="x4")
            x5 = big.tile([P, T * 16], F32, name="x5")
            hh = big.tile([P, T * 16], F32, name="hh")
            h2 = big.tile([P, T * 16], F32, name="h2")
            h3 = big.tile([P, T * 16], F32, name="h3")
            e5 = big.tile([P, T * 16], F32, name="e5")
            env2s = big.tile([P, T * 16], F16, name="env2s")  # 2*env/64 f16
            ki = big.tile([P, T * 16], I32, name="ki")
            kf = big.tile([P, T * 16], F32, name="kf")
            ff = big.tile([P, T * 16], F32, name="ff")  # frac(d/2C)
            uu = big.tile([P, T * 16], F32, name="uu")  # sin(pi f)
            u2 = big.tile([P, T * 16], F32, name="u2")
            tc1h = big.tile([P, T * 16], F16, name="tc1h")  # 2 cos(2pi f)
            s1f = big.tile([P, T * 16], F16, name="s1f")  # sin(2pi f)
            etm = big.tile([P, T * 16], F16, name="etm")  # recurrence tmp
            erbf = big.tile([P, T * 96], F16, name="erbf")  # [t,r,b] (= 2 env sin / 64)
            ch16 = big.tile([P, T * 256], F16, name="ch16")  # cos(a,b) [t,a,b]
            al2 = big.tile([P, T * 256], F16, name="al2")  # alpha/2 [t,a,b]

            # zero the c=3 pad lane of vh16 once
            nc.gpsimd.memset(sub(vh16[:], 3, [[64, T], [4, 16]]), 0.0)

            def SR(buf, s, per, dims):
                return sub(buf[:], s * SCT * per, dims)

            def CR(buf, c, per, dims):
                return sub(buf[:], c * TC * per, dims)

            # =============== prep phases, per superchunk ===============
            for s in range(SC):
                # window DMA
                src = AP(
                    tensor=win,
                    offset=s * SCT * P * 3,
                    ap=[[3, P], [P * 3, SCT], [1, 51]],
                )
                nc.sync.dma_start(SR(w, s, 51, [[1, SCT * 51]]), src)

                # V build (Pool), c-major: V[t,c,b]; b 0..7 <- +1..+8, 8..15 <- -1..-8
                ctr = AP(
                    tensor=w.tensor,
                    offset=w[:].offset + s * SCT * 51 + 24,
                    ap=[list(w[:].ap[0]), [51, SCT], [1, 3], [0, 8]],
                )
                nc.gpsimd.tensor_tensor(
                    SR(vc, s, 48, [[48, SCT], [16, 3], [1, 8]]),
                    AP(
                        tensor=w.tensor,
                        offset=w[:].offset + s * SCT * 51 + 27,
                        ap=[list(w[:].ap[0]), [51, SCT], [1, 3], [3, 8]],
                    ),
                    ctr,
                    ALU.subtract,
                )
                nc.gpsimd.tensor_tensor(
                    sub(vc[:], s * SCT * 48 + 8, [[48, SCT], [16, 3], [1, 8]]),
                    AP(
                        tensor=w.tensor,
                        offset=w[:].offset + s * SCT * 51 + 21,
                        ap=[list(w[:].ap[0]), [51, SCT], [1, 3], [-3, 8]],
                    ),
                    ctr,
                    ALU.subtract,
                )

                # n2 = sum_c V^2: Square on ACT, two adds on DVE
                # vh32 doubles as V^2 scratch before being overwritten by Vhat
                nc.scalar.activation(
                    SR(vh32, s, 48, [[1, SCT * 48]]),
                    SR(vc, s, 48, [[1, SCT * 48]]),
                    ACT.Square,
                )
                nc.vector.tensor_tensor(
                    SR(n2a, s, 16, [[1, SCT * 16]]),
                    SR(vh32, s, 48, [[48, SCT], [1, 16]]),
                    sub(vh32[:], s * SCT * 48 + 16, [[48, SCT], [1, 16]]),
                    ALU.add,
                )
                nc.vector.tensor_tensor(
                    SR(n2, s, 16, [[1, SCT * 16]]),
                    SR(n2a, s, 16, [[1, SCT * 16]]),
                    sub(vh32[:], s * SCT * 48 + 32, [[48, SCT], [1, 16]]),
                    ALU.add,
                )

                # yn = 1/d = exp(-0.5 ln n2)  (ACT, natural_log_exp table)
                nc.scalar.activation(
                    SR(n2a, s, 16, [[1, SCT * 16]]), SR(n2, s, 16, [[1, SCT * 16]]),
                    ACT.Ln,
                )
                nc.scalar.activation(
                    SR(yn, s, 16, [[1, SCT * 16]]), SR(n2a, s, 16, [[1, SCT * 16]]),
                    ACT.Exp, scale=-0.5,
                )

                # x = d/(2C) ; dc = 2x ; Vhat = V * yn
                nc.vector.scalar_tensor_tensor(
                    SR(xh, s, 16, [[1, SCT * 16]]),
                    SR(n2, s, 16, [[1, SCT * 16]]),
                    1.0 / (2.0 * CUTOFF),
                    SR(yn, s, 16, [[1, SCT * 16]]),
                    ALU.mult, ALU.mult,
                )
                nc.vector.tensor_scalar(
                    SR(dcb, s, 16, [[1, SCT * 16]]),
                    SR(xh, s, 16, [[1, SCT * 16]]),
                    2.0, None, ALU.mult,
                )
                nc.vector.tensor_tensor(
                    SR(vh32, s, 48, [[48, SCT], [16, 3], [1, 16]]),
                    SR(vc, s, 48, [[48, SCT], [16, 3], [1, 16]]),
                    SR(yn, s, 16, [[16, SCT], [0, 3], [1, 16]]),
                    ALU.mult,
                )
                # vh16[t,b,c4] (lanes 0..2) <- cast of vh32[t,c,b]  (ACT Copy)
                nc.scalar.activation(
                    SR(vh16, s, 64, [[64, SCT], [4, 16], [1, 3]]),
                    SR(vh32, s, 48, [[48, SCT], [1, 16], [16, 3]]),
                    ACT.Copy,
                )

                # envelope: env2s = YN64*yn + x5*(EA64 + EB64*dc + EC64*dc^2)
                dcs = SR(dcb, s, 16, [[1, SCT * 16]])
                nc.gpsimd.tensor_tensor(SR(qq, s, 16, [[1, SCT * 16]]), dcs, dcs, ALU.mult)
                nc.gpsimd.tensor_tensor(
                    SR(x4, s, 16, [[1, SCT * 16]]),
                    SR(qq, s, 16, [[1, SCT * 16]]),
                    SR(qq, s, 16, [[1, SCT * 16]]),
                    ALU.mult,
                )
                nc.gpsimd.tensor_tensor(
                    SR(x5, s, 16, [[1, SCT * 16]]),
                    SR(x4, s, 16, [[1, SCT * 16]]), dcs, ALU.mult,
                )
                nc.vector.tensor_scalar(
                    SR(hh, s, 16, [[1, SCT * 16]]), dcs, EC64, EB64, ALU.mult, ALU.add
                )
                nc.gpsimd.tensor_tensor(
                    SR(h2, s, 16, [[1, SCT * 16]]),
                    SR(hh, s, 16, [[1, SCT * 16]]), dcs, ALU.mult,
                )
                nc.vector.tensor_scalar(
                    SR(h3, s, 16, [[1, SCT * 16]]),
                    SR(h2, s, 16, [[1, SCT * 16]]), EA64, None, ALU.add,
                )
                nc.gpsimd.tensor_tensor(
                    SR(e5, s, 16, [[1, SCT * 16]]),
                    SR(x5, s, 16, [[1, SCT * 16]]),
                    SR(h3, s, 16, [[1, SCT * 16]]),
                    ALU.mult,
                )
                nc.vector.scalar_tensor_tensor(
                    SR(env2s, s, 16, [[1, SCT * 16]]),
                    SR(yn, s, 16, [[1, SCT * 16]]),
                    YN64,
                    SR(e5, s, 16, [[1, SCT * 16]]),
                    ALU.mult, ALU.add,
                )

                # f = frac(x) via int cast (x >= 0)
                nc.vector.tensor_copy(
                    SR(ki, s, 16, [[1, SCT * 16]]), SR(xh, s, 16, [[1, SCT * 16]])
                )
                nc.vector.tensor_copy(
                    SR(kf, s, 16, [[1, SCT * 16]]), SR(ki, s, 16, [[1, SCT * 16]])
                )
                nc.vector.tensor_tensor(
                    SR(ff, s, 16, [[1, SCT * 16]]),
                    SR(xh, s, 16, [[1, SCT * 16]]),
                    SR(kf, s, 16, [[1, SCT * 16]]),
                    ALU.subtract,
                )

                # u = sin(pi f) ; s1 = sin(2 pi f)  (trig table)
                nc.scalar.activation(
                    SR(uu, s, 16, [[1, SCT * 16]]), SR(ff, s, 16, [[1, SCT * 16]]),
                    ACT.Sin, scale=PI,
                )
                nc.scalar.activation(
                    SR(s1f, s, 16, [[1, SCT * 16]]), SR(ff, s, 16, [[1, SCT * 16]]),
                    ACT.Sin, scale=2.0 * PI,
                )

                # tc1 = 2 cos(2 pi f) = 2 - 4 u^2  (f16)
                nc.vector.tensor_tensor(
                    SR(u2, s, 16, [[1, SCT * 16]]),
                    SR(uu, s, 16, [[1, SCT * 16]]),
                    SR(uu, s, 16, [[1, SCT * 16]]),
                    ALU.mult,
                )
                nc.vector.tensor_scalar(
                    SR(tc1h, s, 16, [[1, SCT * 16]]),
                    SR(u2, s, 16, [[1, SCT * 16]]),
                    -4.0, 2.0, ALU.mult, ALU.add,
                )

                # E recurrence -> erbf[t, r, b], r = n-1, all f16
                def EB(r, dims=None):
                    return sub(erbf[:], s * SCT * 96 + r * 16, [[96, SCT], [1, 16]])

                nc.vector.tensor_tensor(
                    EB(0), SR(env2s, s, 16, [[16, SCT], [1, 16]]),
                    SR(s1f, s, 16, [[16, SCT], [1, 16]]), ALU.mult,
                )
                tcs = SR(tc1h, s, 16, [[16, SCT], [1, 16]])
                nc.vector.tensor_tensor(EB(1), tcs, EB(0), ALU.mult)
                for r in range(2, 6):
                    nc.vector.tensor_tensor(
                        SR(etm, s, 16, [[16, SCT], [1, 16]]), tcs, EB(r - 1), ALU.mult
                    )
                    nc.vector.tensor_tensor(
                        EB(r), SR(etm, s, 16, [[16, SCT], [1, 16]]), EB(r - 2),
                        ALU.subtract,
                    )

            # =============== gram / alpha / contraction, per chunk ===============
            for c in range(CH):
                cb = c * TC  # first tile of chunk
                # ---- Gram on Pool: tiles [cb, cb+KP), f32, one instr per coord ----
                p3 = work.tile([P, KP * 768], F32, tag="p3", bufs=1)
                for cc in range(3):
                    nc.gpsimd.tensor_tensor(
                        sub(p3[:], cc * KP * 256, [[256, KP], [16, 16], [1, 16]]),
                        sub(vh32[:], cb * 48 + cc * 16, [[48, KP], [1, 16], [0, 16]]),
                        sub(vh32[:], cb * 48 + cc * 16, [[48, KP], [0, 16], [1, 16]]),
                        ALU.mult,
                    )
                nc.gpsimd.tensor_tensor(
                    sub(p3[:], 0, [[256, KP], [1, 256]]),
                    sub(p3[:], 0, [[256, KP], [1, 256]]),
                    sub(p3[:], KP * 256, [[256, KP], [1, 256]]),
                    ALU.add,
                )
                nc.gpsimd.tensor_tensor(
                    sub(ch16[:], cb * 256, [[256, KP], [1, 256]]),
                    sub(p3[:], 0, [[256, KP], [1, 256]]),
                    sub(p3[:], 2 * KP * 256, [[256, KP], [1, 256]]),
                    ALU.add,
                )
                # ---- Gram on DVE: tiles [cb+KP, cb+8), f16 c4 layout ----
                if KD > 0:
                    p16 = work.tile([P, KD * 1024], F16, tag="p16", bufs=2)
                    sg = work.tile([P, KD * 512], F16, tag="sg", bufs=2)
                    for g in range(KD):
                        t = cb + KP + g
                        nc.vector.tensor_tensor(
                            sub(p16[:], g * 1024, [[64, 16], [4, 16], [1, 4]]),
                            sub(vh16[:], t * 64, [[4, 16], [0, 16], [1, 4]]),
                            sub(vh16[:], t * 64, [[0, 16], [4, 16], [1, 4]]),
                            ALU.mult,
                        )
                    nc.vector.tensor_tensor(
                        sub(sg[:], 0, [[512, KD], [2, 256], [1, 2]]),
                        sub(p16[:], 0, [[1024, KD], [4, 256], [1, 2]]),
                        sub(p16[:], 2, [[1024, KD], [4, 256], [1, 2]]),
                        ALU.add,
                    )
                    nc.vector.tensor_tensor(
                        sub(ch16[:], (cb + KP) * 256, [[256, KD], [1, 256]]),
                        sub(sg[:], 0, [[512, KD], [2, 256]]),
                        sub(sg[:], 1, [[512, KD], [2, 256]]),
                        ALU.add,
                    )

                # ---- clamp to [-1, 1] ----
                nc.vector.tensor_scalar(
                    CR(ch16, c, 256, [[1, TC * 256]]),
                    CR(ch16, c, 256, [[1, TC * 256]]),
                    -1.0, 1.0, ALU.max, ALU.min,
                )

                # ---- alpha/2 = atan(exp(0.5 (ln((1-c)/2) - ln((1+c)/2)))) ----
                lnA = work.tile([P, TC * 256], F16, tag="lnA", bufs=1)
                lnB = work.tile([P, TC * 256], F16, tag="lnB", bufs=1)
                dln = work.tile([P, TC * 256], F16, tag="dln", bufs=1)
                tq = work.tile([P, TC * 256], F16, tag="tq", bufs=1)
                nc.scalar.activation(
                    lnA[:], CR(ch16, c, 256, [[1, TC * 256]]), ACT.Ln,
                    bias=0.5, scale=-0.5,
                )
                nc.scalar.activation(
                    lnB[:], CR(ch16, c, 256, [[1, TC * 256]]), ACT.Ln,
                    bias=0.5, scale=0.5,
                )
                nc.vector.tensor_tensor(dln[:], lnA[:], lnB[:], ALU.subtract)
                nc.scalar.activation(tq[:], dln[:], ACT.Exp, scale=0.5)
                nc.scalar.activation(
                    CR(al2, c, 256, [[1, TC * 256]]), tq[:], ACT.Arctan
                )
                # diagonal alpha := 0 (i != k exclusion)
                nc.gpsimd.memset(
                    sub(al2[:], cb * 256, [[256, TC], [17, 16]]), 0.0
                )

                # ---- contraction: ot[t,a,r] = sum_b al2[t,a,b] * erbf[t,r,b] ----
                p4 = work.tile([P, TC * 1536], F16, tag="p4", bufs=1)
                t1 = work.tile([P, TC * 768], F16, tag="t1", bufs=1)
                t2 = work.tile([P, TC * 384], F16, tag="t2", bufs=1)
                t3 = work.tile([P, TC * 192], F16, tag="t3", bufs=1)
                otc = work.tile([P, TC * 96], F32, tag="otc", bufs=2)
                for tl in range(TC):
                    t = cb + tl
                    nc.vector.tensor_tensor(
                        sub(p4[:], tl * 1536, [[96, 16], [16, 6], [1, 16]]),
                        sub(al2[:], t * 256, [[16, 16], [0, 6], [1, 16]]),
                        sub(erbf[:], t * 96, [[0, 16], [16, 6], [1, 16]]),
                        ALU.mult,
                    )
                nc.vector.tensor_tensor(
                    sub(t1[:], 0, [[768, TC], [8, 96], [1, 8]]),
                    sub(p4[:], 0, [[1536, TC], [16, 96], [1, 8]]),
                    sub(p4[:], 8, [[1536, TC], [16, 96], [1, 8]]),
                    ALU.add,
                )
                nc.vector.tensor_tensor(
                    sub(t2[:], 0, [[384, TC], [4, 96], [1, 4]]),
                    sub(t1[:], 0, [[768, TC], [8, 96], [1, 4]]),
                    sub(t1[:], 4, [[768, TC], [8, 96], [1, 4]]),
                    ALU.add,
                )
                nc.vector.tensor_tensor(
                    sub(t3[:], 0, [[192, TC], [2, 96], [1, 2]]),
                    sub(t2[:], 0, [[384, TC], [4, 96], [1, 2]]),
                    sub(t2[:], 2, [[384, TC], [4, 96], [1, 2]]),
                    ALU.add,
                )
                nc.vector.tensor_tensor(
                    sub(otc[:], 0, [[1, TC * 96]]),
                    sub(t3[:], 0, [[2, TC * 96]]),
                    sub(t3[:], 1, [[2, TC * 96]]),
                    ALU.add,
                )
                nc.vector.tensor_scalar(
                    sub(otc[:], 0, [[1, TC * 96]]),
                    sub(otc[:], 0, [[1, TC * 96]]),
                    OSCALE, None, ALU.mult,
                )
                # out rows (t*128 + p)*16 + a, cols r  (one DMA per chunk)
                dst = AP(
                    tensor=out,
                    offset=c * TC * P * 96,
                    ap=[[96, P], [96 * P, TC], [1, 96]],
                )
                nc.sync.dma_start(dst, sub(otc[:], 0, [[96, TC], [1, 96]]))

    nc.compile()
    return nc


def _get_nc():
    global _cached_nc
    if _cached_nc is None:
        _cached_nc = _build_device_kernel()
    return _cached_nc


def _make_consts():
    ncv = np.zeros(8, np.float32)
    ncv[0] = 0.5
    return np.broadcast_to(ncv, (P, 8)).copy()


def _run_device(xyz, trace=False, tmpdir=None):
    from concourse import bass_utils

    nc = _get_nc()
    consts = _make_consts()
    ext = np.concatenate([xyz[-HALF:], xyz, xyz[:HALF]], axis=0)  # halo-extended
    in_maps = []
    for c in range(N_CORES):
        base = c * J_PER_CORE
        winc = np.ascontiguousarray(ext[base : base + WIN_ROWS])
        in_maps.append({"win": winc, "consts": consts})
    kwargs = {}
    if trace:
        kwargs = dict(trace=True)
        if tmpdir is not None:
            kwargs["tmpdir"] = tmpdir
    res = bass_utils.run_bass_kernel_spmd(
        nc, in_maps, core_ids=list(range(N_CORES)), **kwargs
    )
    shards = [res.results[c]["out"] for c in range(N_CORES)]
    full = np.concatenate(shards, axis=0).astype(np.float32)
    return full, res


def kernel(xyz, nbr_list, angle_list):
    xyz = np.asarray(xyz, dtype=np.float32)
    if not _graph_matches(nbr_list, angle_list):
        return _fallback_numpy(xyz, nbr_list, angle_list)
    out, _ = _run_device(xyz)
    return out


# revision 10
# speedup vs baseline: 1.5824x; 1.2764x over previous
"""Bass/Trainium2 kernel for nn_DimeNet_22737556865501.

Strategy (v3)
-------------
Circulant-structure collapse (per-atom dense math on the 16 local
displacement vectors), restructured from v2 around trace findings:

- GpSimd (Pool) runs ~2.2-3 ns/elem vs DVE 0.52 (f16 2x) / 1.04 (f32):
  shift the bulk to DVE in fp16, keep Pool on V-build/envelope/partial
  Gram (f32, precision anchor).
- fp16 (not bf16) for alpha/erbf/contraction: 8x lower rounding error,
  2x DVE mode confirmed (incl. outer-broadcast APs). erbf carries a
  1/64 scale to stay in f16 range; the final f32 stage multiplies by 64.
- e_rbf sin harmonics via the scale-invariant Chebyshev recurrence
  E_n = 2cos(theta) E_{n-1} - E_{n-2} seeded from two in-range Sin
  calls (args in [0, 2pi)); kills the per-harmonic range-reduction
  casts of v2.
- alpha chain: clamp, Ln((1-c)/2), Ln((1+c)/2), f16 subtract,
  Exp(0.5 dln) = tan(alpha/2), Arctan -> alpha/2 in f16. Two ACT
  tables total (natural_log_exp + trig).
- Gram cos matrix split by tile: K_POOL tiles/chunk on Pool (f32,
  3 chunk-wide product instrs, one per coordinate) and the rest on DVE
  (f16, per-tile products over a c4-padded layout, 2x mode).

Sharding: atoms partitioned across the 8 NeuronCores (4096 each); each
core writes its own 65536x6 output rows; host concatenates. Host
verifies the circulant graph and falls back to exact numpy otherwise.
"""

import numpy as np

N_ATOMS = 32768
DEG = 16
HALF = DEG // 2
N_CORES = 8
J_PER_CORE = N_ATOMS // N_CORES  # 4096
P = 128  # partitions / atoms per tile
T = J_PER_CORE // P  # 32 tiles
CH = 4  # chunks (gram/alpha/contraction granularity)
TC = T // CH  # 8 tiles per chunk
SC = 2  # superchunks (prep granularity)
SCT = T // SC  # 16 tiles per superchunk
WIN_ROWS = J_PER_CORE + DEG  # 4112 (8-row halo each side)
N_RBF = 6
CUTOFF = 5.0
ENV_P = 6
A_ = -(ENV_P + 1) * (ENV_P + 2) / 2.0  # -28
B_ = float(ENV_P * (ENV_P + 2))  # 48
C_ = -ENV_P * (ENV_P + 1) / 2.0  # -21
OSCALE = 64.0  # erbf carries 1/OSCALE; final f32 stage multiplies back
EA64 = 2.0 * A_ / OSCALE
EB64 = 2.0 * B_ / OSCALE
EC64 = 2.0 * C_ / OSCALE
YN64 = 2.0 * CUTOFF / OSCALE  # (2/64) * (1/dc) = YN64 * yn
PI = float(np.pi)

# v4: GpSimd (Pool) and DVE share an SBUF port pair with an exclusive
# lock -- every DVE tensor_tensor blocks while a Pool op runs, so Pool
# time is serial with DVE time. Keep Pool idle; ACT has its own port
# and truly overlaps.
K_POOL = 0  # all gram tiles on DVE (f16)

_cached_nc = None


def _expected_graph():
    half = HALF
    offs = np.concatenate([np.arange(1, half + 1), N_ATOMS - np.arange(1, half + 1)])
    j = np.arange(N_ATOMS)
    nbr_dst = (j[:, None] + offs[None, :]) % N_ATOMS
    nbr_list = np.stack([np.repeat(j, DEG), nbr_dst.reshape(-1)], 1)
    o1, o2 = np.meshgrid(offs, offs, indexing="ij")
    keep = o1 != o2
    o1, o2 = o1[keep], o2[keep]
    i = (j[:, None] + o1[None, :]) % N_ATOMS
    k = (j[:, None] + o2[None, :]) % N_ATOMS
    jc = np.broadcast_to(j[:, None], i.shape)
    angle_list = np.stack([i.reshape(-1), jc.reshape(-1), k.reshape(-1)], 1)
    return nbr_list.astype(np.int64), angle_list.astype(np.int64)


def _graph_matches(nbr_list, angle_list):
    if nbr_list.shape != (N_ATOMS * DEG, 2):
        return False
    if angle_list.shape != (N_ATOMS * DEG * (DEG - 1), 3):
        return False
    exp_nbr, exp_ang = _expected_graph()
    return np.array_equal(np.asarray(nbr_list), exp_nbr) and np.array_equal(
        np.asarray(angle_list), exp_ang
    )


def _fallback_numpy(xyz, nbr_list, angle_list):
    """Exact numpy mirror of the jax reference (general graph)."""
    xyz = np.asarray(xyz, dtype=np.float32)
    nbr = np.asarray(nbr_list)
    ang = np.asarray(angle_list)
    E = nbr.shape[0]
    r_ji = xyz[ang[:, 0]] - xyz[ang[:, 1]]
    r_jk = xyz[ang[:, 2]] - xyz[ang[:, 1]]
    dot = np.sum(r_ji * r_jk, axis=-1)
    crs = np.linalg.norm(np.cross(r_ji, r_jk), axis=-1)
    alpha = np.arctan2(crs, dot)
    diff = xyz[nbr[:, 0]] - xyz[nbr[:, 1]]
    d = np.linalg.norm(diff, axis=-1)
    n = np.arange(1, N_RBF + 1, dtype=xyz.dtype)
    dc = (d / CUTOFF)[:, None]
    env = 1.0 / dc + A_ * dc ** (ENV_P - 1) + B_ * dc**ENV_P + C_ * dc ** (ENV_P + 1)
    e_rbf = env * np.sin(n * np.pi * dc)
    keys = nbr[:, 0] * N_ATOMS + nbr[:, 1]
    order = np.argsort(keys, kind="stable")
    ji_idx = order[np.searchsorted(keys[order], ang[:, 1] * N_ATOMS + ang[:, 0])]
    kj_idx = order[np.searchsorted(keys[order], ang[:, 2] * N_ATOMS + ang[:, 1])]
    trip = alpha[:, None] * e_rbf[kj_idx]
    out = np.zeros((E, N_RBF), dtype=np.float32)
    np.add.at(out, ji_idx, trip.astype(np.float32))
    return out


# ---------------------------------------------------------------------------
# Device kernel
# ---------------------------------------------------------------------------


def _build_device_kernel():
    import concourse.bacc as bacc
    import concourse.mybir as mybir
    from concourse.bass_types import AP
    from concourse.tile import TileContext

    F32 = mybir.dt.float32
    F16 = mybir.dt.float16
    I32 = mybir.dt.int32
    ALU = mybir.AluOpType
    ACT = mybir.ActivationFunctionType

    KP = K_POOL
    KD = TC - KP  # DVE gram tiles per chunk

    nc = bacc.Bacc("TRN2", target_bir_lowering=False, debug=False, num_devices=N_CORES)
    win = nc.dram_tensor("win", [WIN_ROWS, 3], F32, kind="ExternalInput")
    consts = nc.dram_tensor("consts", [P, 8], F32, kind="ExternalInput")
    out = nc.dram_tensor("out", [J_PER_CORE * DEG, N_RBF], F32, kind="ExternalOutput")

    def sub(base: AP, off: int, dims) -> AP:
        """Sub-AP of an SBUF tile: keep partition dim, custom free dims."""
        return AP(
            tensor=base.tensor,
            offset=base.offset + off,
            ap=[list(base.ap[0]), *[list(d) for d in dims]],
        )

    with TileContext(nc) as tc:
        with (
            tc.tile_pool(name="big", bufs=1) as big,
            tc.tile_pool(name="work", bufs=2) as work,
        ):
            nco = big.tile([P, 8], F32, name="nco")
            nc.sync.dma_start(nco[:], consts[:])
            # register 0.5 (slot 0) for activation bias use
            nc.const_aps.aps[(F32, 0.5)] = sub(nco[:], 0, [[1, 1]])

            # ---- persistent per-core buffers (free sizes per partition) ----
            w = big.tile([P, T * 51], F32, name="w")
            vc = big.tile([P, T * 48], F32, name="vc")  # V [t,c,b] f32
            n2a = big.tile([P, T * 16], F32, name="n2a")
            n2 = big.tile([P, T * 16], F32, name="n2")
            yn = big.tile([P, T * 16], F32, name="yn")  # 1/d
            xh = big.tile([P, T * 16], F32, name="xh")  # d/(2C)
            dcb = big.tile([P, T * 16], F32, name="dcb")  # d/C
            vh32 = big.tile([P, T * 48], F32, name="vh32")  # Vhat [t,c,b] f32
            vh16 = big.tile([P, T * 64], F16, name="vh16")  # Vhat [t,b,c4] f16
            qq = big.tile([P, T * 16], F32, name="qq")
            x4 = big.tile([P, T * 16], F32, name="x4")
            x5 = big.tile([P, T * 16], F32, name="x5")
            hh = big.tile([P, T * 16], F32, name="hh")
            h2 = big.tile([P, T * 16], F32, name="h2")
            h3 = big.tile([P, T * 16], F32, name="h3")
            e5 = big.tile([P, T * 16], F32, name="e5")
            env2s = big.tile([P, T * 16], F16, name="env2s")  # 2*env/64 f16
            ki = big.tile([P, T * 16], I32, name="ki")
            kf = big.tile([P, T * 16], F32, name="kf")
            ff = big.tile([P, T * 16], F32, name="ff")  # frac(d/2C)
            uu = big.tile([P, T * 16], F32, name="uu")  # sin(pi f)
            u2 = big.tile([P, T * 16], F32, name="u2")
            tc1h = big.tile([P, T * 16], F16, name="tc1h")  # 2 cos(2pi f)
            s1f = big.tile([P, T * 16], F16, name="s1f")  # sin(2pi f)
            etm = big.tile([P, T * 16], F16, name="etm")  # recurrence tmp
            erbf = big.tile([P, T * 96], F16, name="erbf")  # [t,r,b] (= 2 env sin / 64)
            ch16 = big.tile([P, T * 256], F16, name="ch16")  # cos(a,b) [t,a,b]
            al2 = big.tile([P, T * 256], F16, name="al2")  # alpha/2 [t,a,b]

            # zero the c=3 pad lane of vh16 once
            nc.gpsimd.memset(sub(vh16[:], 3, [[64, T], [4, 16]]), 0.0)

            def SR(buf, s, per, dims):
                return sub(buf[:], s * SCT * per, dims)

            def CR(buf, c, per, dims):
                return sub(buf[:], c * TC * per, dims)

            # =============== prep phases, per superchunk ===============
            for s in range(SC):
                # window DMA
                src = AP(
                    tensor=win,
                    offset=s * SCT * P * 3,
                    ap=[[3, P], [P * 3, SCT], [1, 51]],
                )
                nc.sync.dma_start(SR(w, s, 51, [[1, SCT * 51]]), src)

                # V build (DVE), c-major: V[t,c,b]; b 0..7 <- +1..+8, 8..15 <- -1..-8
                ctr = AP(
                    tensor=w.tensor,
                    offset=w[:].offset + s * SCT * 51 + 24,
                    ap=[list(w[:].ap[0]), [51, SCT], [1, 3], [0, 8]],
                )
                nc.vector.tensor_tensor(
                    SR(vc, s, 48, [[48, SCT], [16, 3], [1, 8]]),
                    AP(
                        tensor=w.tensor,
                        offset=w[:].offset + s * SCT * 51 + 27,
                        ap=[list(w[:].ap[0]), [51, SCT], [1, 3], [3, 8]],
                    ),
                    ctr,
                    ALU.subtract,
                )
                nc.vector.tensor_tensor(
                    sub(vc[:], s * SCT * 48 + 8, [[48, SCT], [16, 3], [1, 8]]),
                    AP(
                        tensor=w.tensor,
                        offset=w[:].offset + s * SCT * 51 + 21,
                        ap=[list(w[:].ap[0]), [51, SCT], [1, 3], [-3, 8]],
                    ),
                    ctr,
                    ALU.subtract,
                )

                # n2 = sum_c V^2: Square on ACT, two adds on DVE
                # vh32 doubles as V^2 scratch before being overwritten by Vhat
                nc.scalar.activation(
                    SR(vh32, s, 48, [[1, SCT * 48]]),
                    SR(vc, s, 48, [[1, SCT * 48]]),
                    ACT.Square,
                )
                nc.vector.tensor_tensor(
                    SR(n2a, s, 16, [[1, SCT * 16]]),
                    SR(vh32, s, 48, [[48, SCT], [1, 16]]),
                    sub(vh32[:], s * SCT * 48 + 16, [[48, SCT], [1, 16]]),
                    ALU.add,
                )
                nc.vector.tensor_tensor(
                    SR(n2, s, 16, [[1, SCT * 16]]),
                    SR(n2a, s, 16, [[1, SCT * 16]]),
                    sub(vh32[:], s * SCT * 48 + 32, [[48, SCT], [1, 16]]),
                    ALU.add,
                )

                # yn = 1/d = exp(-0.5 ln n2)  (ACT, natural_log_exp table)
                nc.scalar.activation(
                    SR(n2a, s, 16, [[1, SCT * 16]]), SR(n2, s, 16, [[1, SCT * 16]]),
                    ACT.Ln,
                )
                nc.scalar.activation(
                    SR(yn, s, 16, [[1, SCT * 16]]), SR(n2a, s, 16, [[1, SCT * 16]]),
                    ACT.Exp, scale=-0.5,
                )

                # x = d/(2C) ; dc = 2x ; Vhat = V * yn
                nc.vector.scalar_tensor_tensor(
                    SR(xh, s, 16, [[1, SCT * 16]]),
                    SR(n2, s, 16, [[1, SCT * 16]]),
                    1.0 / (2.0 * CUTOFF),
                    SR(yn, s, 16, [[1, SCT * 16]]),
                    ALU.mult, ALU.mult,
                )
                nc.vector.tensor_scalar(
                    SR(dcb, s, 16, [[1, SCT * 16]]),
                    SR(xh, s, 16, [[1, SCT * 16]]),
                    2.0, None, ALU.mult,
                )
                nc.vector.tensor_tensor(
                    SR(vh32, s, 48, [[48, SCT], [16, 3], [1, 16]]),
                    SR(vc, s, 48, [[48, SCT], [16, 3], [1, 16]]),
                    SR(yn, s, 16, [[16, SCT], [0, 3], [1, 16]]),
                    ALU.mult,
                )
                # vh16[t,b,c4] (lanes 0..2) <- cast of vh32[t,c,b]  (ACT Copy)
                nc.scalar.activation(
                    SR(vh16, s, 64, [[64, SCT], [4, 16], [1, 3]]),
                    SR(vh32, s, 48, [[48, SCT], [1, 16], [16, 3]]),
                    ACT.Copy,
                )

                # envelope: env2s = YN64*yn + x5*(EA64 + EB64*dc + EC64*dc^2)
                dcs = SR(dcb, s, 16, [[1, SCT * 16]])
                nc.scalar.activation(SR(qq, s, 16, [[1, SCT * 16]]), dcs, ACT.Square)
                nc.scalar.activation(
                    SR(x4, s, 16, [[1, SCT * 16]]),
                    SR(qq, s, 16, [[1, SCT * 16]]), ACT.Square,
                )
                nc.vector.tensor_tensor(
                    SR(x5, s, 16, [[1, SCT * 16]]),
                    SR(x4, s, 16, [[1, SCT * 16]]), dcs, ALU.mult,
                )
                nc.vector.tensor_scalar(
                    SR(hh, s, 16, [[1, SCT * 16]]), dcs, EC64, EB64, ALU.mult, ALU.add
                )
                nc.vector.tensor_tensor(
                    SR(h2, s, 16, [[1, SCT * 16]]),
                    SR(hh, s, 16, [[1, SCT * 16]]), dcs, ALU.mult,
                )
                nc.vector.tensor_scalar(
                    SR(h3, s, 16, [[1, SCT * 16]]),
                    SR(h2, s, 16, [[1, SCT * 16]]), EA64, None, ALU.add,
                )
                nc.vector.tensor_tensor(
                    SR(e5, s, 16, [[1, SCT * 16]]),
                    SR(x5, s, 16, [[1, SCT * 16]]),
                    SR(h3, s, 16, [[1, SCT * 16]]),
                    ALU.mult,
                )
                nc.vector.scalar_tensor_tensor(
                    SR(env2s, s, 16, [[1, SCT * 16]]),
                    SR(yn, s, 16, [[1, SCT * 16]]),
                    YN64,
                    SR(e5, s, 16, [[1, SCT * 16]]),
                    ALU.mult, ALU.add,
                )

                # f = frac(x) via int cast (x >= 0)
                nc.vector.tensor_copy(
                    SR(ki, s, 16, [[1, SCT * 16]]), SR(xh, s, 16, [[1, SCT * 16]])
                )
                nc.vector.tensor_copy(
                    SR(kf, s, 16, [[1, SCT * 16]]), SR(ki, s, 16, [[1, SCT * 16]])
                )
                nc.vector.tensor_tensor(
                    SR(ff, s, 16, [[1, SCT * 16]]),
                    SR(xh, s, 16, [[1, SCT * 16]]),
                    SR(kf, s, 16, [[1, SCT * 16]]),
                    ALU.subtract,
                )

                # u = sin(pi f) ; s1 = sin(2 pi f)  (trig table)
                nc.scalar.activation(
                    SR(uu, s, 16, [[1, SCT * 16]]), SR(ff, s, 16, [[1, SCT * 16]]),
                    ACT.Sin, scale=PI,
                )
                nc.scalar.activation(
                    SR(s1f, s, 16, [[1, SCT * 16]]), SR(ff, s, 16, [[1, SCT * 16]]),
                    ACT.Sin, scale=2.0 * PI,
                )

                # tc1 = 2 cos(2 pi f) = 2 - 4 u^2  (f16)
                nc.scalar.activation(
                    SR(u2, s, 16, [[1, SCT * 16]]),
                    SR(uu, s, 16, [[1, SCT * 16]]),
                    ACT.Square,
                )
                nc.vector.tensor_scalar(
                    SR(tc1h, s, 16, [[1, SCT * 16]]),
                    SR(u2, s, 16, [[1, SCT * 16]]),
                    -4.0, 2.0, ALU.mult, ALU.add,
                )

                # E recurrence -> erbf[t, r, b], r = n-1, all f16
                def EB(r, dims=None):
                    return sub(erbf[:], s * SCT * 96 + r * 16, [[96, SCT], [1, 16]])

                nc.vector.tensor_tensor(
                    EB(0), SR(env2s, s, 16, [[16, SCT], [1, 16]]),
                    SR(s1f, s, 16, [[16, SCT], [1, 16]]), ALU.mult,
                )
                tcs = SR(tc1h, s, 16, [[16, SCT], [1, 16]])
                nc.vector.tensor_tensor(EB(1), tcs, EB(0), ALU.mult)
                for r in range(2, 6):
                    nc.vector.tensor_tensor(
                        SR(etm, s, 16, [[16, SCT], [1, 16]]), tcs, EB(r - 1), ALU.mult
                    )
                    nc.vector.tensor_tensor(
                        EB(r), SR(etm, s, 16, [[16, SCT], [1, 16]]), EB(r - 2),
                        ALU.subtract,
                    )

            # =============== gram / alpha / contraction, per chunk ===============
            for c in range(CH):
                cb = c * TC  # first tile of chunk
                # ---- Gram on DVE: all TC tiles, f16 c4 layout ----
                p16 = work.tile([P, KD * 1024], F16, tag="p16", bufs=1)
                sg = work.tile([P, KD * 512], F16, tag="sg", bufs=1)
                for g in range(KD):
                    t = cb + KP + g
                    nc.vector.tensor_tensor(
                        sub(p16[:], g * 1024, [[64, 16], [4, 16], [1, 4]]),
                        sub(vh16[:], t * 64, [[4, 16], [0, 16], [1, 4]]),
                        sub(vh16[:], t * 64, [[0, 16], [4, 16], [1, 4]]),
                        ALU.mult,
                    )
                nc.vector.tensor_tensor(
                    sub(sg[:], 0, [[512, KD], [2, 256], [1, 2]]),
                    sub(p16[:], 0, [[1024, KD], [4, 256], [1, 2]]),
                    sub(p16[:], 2, [[1024, KD], [4, 256], [1, 2]]),
                    ALU.add,
                )
                nc.vector.tensor_tensor(
                    sub(ch16[:], (cb + KP) * 256, [[256, KD], [1, 256]]),
                    sub(sg[:], 0, [[512, KD], [2, 256]]),
                    sub(sg[:], 1, [[512, KD], [2, 256]]),
                    ALU.add,
                )

                # ---- clamp to [-1, 1] ----
                nc.vector.tensor_scalar(
                    CR(ch16, c, 256, [[1, TC * 256]]),
                    CR(ch16, c, 256, [[1, TC * 256]]),
                    -1.0, 1.0, ALU.max, ALU.min,
                )

                # ---- alpha/2 = atan(exp(0.5 (ln((1-c)/2) - ln((1+c)/2)))) ----
                lnA = work.tile([P, TC * 256], F16, tag="lnA", bufs=1)
                lnB = work.tile([P, TC * 256], F16, tag="lnB", bufs=1)
                dln = work.tile([P, TC * 256], F16, tag="dln", bufs=1)
                tq = work.tile([P, TC * 256], F16, tag="tq", bufs=1)
                nc.scalar.activation(
                    lnA[:], CR(ch16, c, 256, [[1, TC * 256]]), ACT.Ln,
                    bias=0.5, scale=-0.5,
                )
                nc.scalar.activation(
                    lnB[:], CR(ch16, c, 256, [[1, TC * 256]]), ACT.Ln,
                    bias=0.5, scale=0.5,
                )
                nc.vector.tensor_tensor(dln[:], lnA[:], lnB[:], ALU.subtract)
                nc.scalar.activation(tq[:], dln[:], ACT.Exp, scale=0.5)
                nc.scalar.activation(
                    CR(al2, c, 256, [[1, TC * 256]]), tq[:], ACT.Arctan
                )
                # diagonal alpha := 0 (i != k exclusion)
                dg = sub(al2[:], cb * 256, [[256, TC], [17, 16]])
                nc.vector.tensor_scalar_mul(dg, dg, 0.0)

                # ---- contraction: ot[t,a,r] = sum_b al2[t,a,b] * erbf[t,r,b] ----
                p4 = work.tile([P, TC * 1536], F16, tag="p4", bufs=1)
                t1 = work.tile([P, TC * 768], F16, tag="t1", bufs=1)
                t2 = work.tile([P, TC * 384], F16, tag="t2", bufs=1)
                t3 = work.tile([P, TC * 192], F16, tag="t3", bufs=1)
                otc = work.tile([P, TC * 96], F32, tag="otc", bufs=2)
                for tl in range(TC):
                    t = cb + tl
                    nc.vector.tensor_tensor(
                        sub(p4[:], tl * 1536, [[96, 16], [16, 6], [1, 16]]),
                        sub(al2[:], t * 256, [[16, 16], [0, 6], [1, 16]]),
                        sub(erbf[:], t * 96, [[0, 16], [16, 6], [1, 16]]),
                        ALU.mult,
                    )
                nc.vector.tensor_tensor(
                    sub(t1[:], 0, [[768, TC], [8, 96], [1, 8]]),
                    sub(p4[:], 0, [[1536, TC], [16, 96], [1, 8]]),
                    sub(p4[:], 8, [[1536, TC], [16, 96], [1, 8]]),
                    ALU.add,
                )
                nc.vector.tensor_tensor(
                    sub(t2[:], 0, [[384, TC], [4, 96], [1, 4]]),
                    sub(t1[:], 0, [[768, TC], [8, 96], [1, 4]]),
                    sub(t1[:], 4, [[768, TC], [8, 96], [1, 4]]),
                    ALU.add,
                )
                nc.vector.tensor_tensor(
                    sub(t3[:], 0, [[192, TC], [2, 96], [1, 2]]),
                    sub(t2[:], 0, [[384, TC], [4, 96], [1, 2]]),
                    sub(t2[:], 2, [[384, TC], [4, 96], [1, 2]]),
                    ALU.add,
                )
                nc.vector.tensor_tensor(
                    sub(otc[:], 0, [[1, TC * 96]]),
                    sub(t3[:], 0, [[2, TC * 96]]),
                    sub(t3[:], 1, [[2, TC * 96]]),
                    ALU.add,
                )
                nc.vector.tensor_scalar(
                    sub(otc[:], 0, [[1, TC * 96]]),
                    sub(otc[:], 0, [[1, TC * 96]]),
                    OSCALE, None, ALU.mult,
                )
                # out rows (t*128 + p)*16 + a, cols r  (one DMA per chunk)
                dst = AP(
                    tensor=out,
                    offset=c * TC * P * 96,
                    ap=[[96, P], [96 * P, TC], [1, 96]],
                )
                nc.sync.dma_start(dst, sub(otc[:], 0, [[96, TC], [1, 96]]))

    nc.compile()
    return nc


def _get_nc():
    global _cached_nc
    if _cached_nc is None:
        _cached_nc = _build_device_kernel()
    return _cached_nc


def _make_consts():
    ncv = np.zeros(8, np.float32)
    ncv[0] = 0.5
    return np.broadcast_to(ncv, (P, 8)).copy()


def _run_device(xyz, trace=False, tmpdir=None):
    from concourse import bass_utils

    nc = _get_nc()
    consts = _make_consts()
    ext = np.concatenate([xyz[-HALF:], xyz, xyz[:HALF]], axis=0)  # halo-extended
    in_maps = []
    for c in range(N_CORES):
        base = c * J_PER_CORE
        winc = np.ascontiguousarray(ext[base : base + WIN_ROWS])
        in_maps.append({"win": winc, "consts": consts})
    kwargs = {}
    if trace:
        kwargs = dict(trace=True)
        if tmpdir is not None:
            kwargs["tmpdir"] = tmpdir
    res = bass_utils.run_bass_kernel_spmd(
        nc, in_maps, core_ids=list(range(N_CORES)), **kwargs
    )
    shards = [res.results[c]["out"] for c in range(N_CORES)]
    full = np.concatenate(shards, axis=0).astype(np.float32)
    return full, res


def kernel(xyz, nbr_list, angle_list):
    xyz = np.asarray(xyz, dtype=np.float32)
    if not _graph_matches(nbr_list, angle_list):
        return _fallback_numpy(xyz, nbr_list, angle_list)
    out, _ = _run_device(xyz)
    return out


# revision 23
# speedup vs baseline: 1.7403x; 1.0998x over previous
"""Bass/Trainium2 kernel for nn_DimeNet_22737556865501.

Strategy (v3)
-------------
Circulant-structure collapse (per-atom dense math on the 16 local
displacement vectors), restructured from v2 around trace findings:

- GpSimd (Pool) runs ~2.2-3 ns/elem vs DVE 0.52 (f16 2x) / 1.04 (f32):
  shift the bulk to DVE in fp16, keep Pool on V-build/envelope/partial
  Gram (f32, precision anchor).
- fp16 (not bf16) for alpha/erbf/contraction: 8x lower rounding error,
  2x DVE mode confirmed (incl. outer-broadcast APs). erbf carries a
  1/64 scale to stay in f16 range; the final f32 stage multiplies by 64.
- e_rbf sin harmonics via the scale-invariant Chebyshev recurrence
  E_n = 2cos(theta) E_{n-1} - E_{n-2} seeded from two in-range Sin
  calls (args in [0, 2pi)); kills the per-harmonic range-reduction
  casts of v2.
- alpha chain: clamp, Ln((1-c)/2), Ln((1+c)/2), f16 subtract,
  Exp(0.5 dln) = tan(alpha/2), Arctan -> alpha/2 in f16. Two ACT
  tables total (natural_log_exp + trig).
- Gram cos matrix split by tile: K_POOL tiles/chunk on Pool (f32,
  3 chunk-wide product instrs, one per coordinate) and the rest on DVE
  (f16, per-tile products over a c4-padded layout, 2x mode).

Sharding: atoms partitioned across the 8 NeuronCores (4096 each); each
core writes its own 65536x6 output rows; host concatenates. Host
verifies the circulant graph and falls back to exact numpy otherwise.
"""

import numpy as np

N_ATOMS = 32768
DEG = 16
HALF = DEG // 2
N_CORES = 8
J_PER_CORE = N_ATOMS // N_CORES  # 4096
P = 128  # partitions / atoms per tile
T = J_PER_CORE // P  # 32 tiles
CH = 4  # chunks (gram/alpha/contraction granularity)
TC = T // CH  # 8 tiles per chunk
SC = 2  # superchunks (prep granularity)
SCT = T // SC  # 16 tiles per superchunk
WIN_ROWS = J_PER_CORE + DEG  # 4112 (8-row halo each side)
N_RBF = 6
CUTOFF = 5.0
ENV_P = 6
A_ = -(ENV_P + 1) * (ENV_P + 2) / 2.0  # -28
B_ = float(ENV_P * (ENV_P + 2))  # 48
C_ = -ENV_P * (ENV_P + 1) / 2.0  # -21
OSCALE = 64.0  # erbf carries 1/OSCALE; final f32 stage multiplies back
EA64 = 2.0 * A_ / OSCALE
EB64 = 2.0 * B_ / OSCALE
EC64 = 2.0 * C_ / OSCALE
YN64 = 2.0 * CUTOFF / OSCALE  # (2/64) * (1/dc) = YN64 * yn
PI = float(np.pi)

# v4: GpSimd (Pool) and DVE share an SBUF port pair with an exclusive
# lock -- every DVE tensor_tensor blocks while a Pool op runs, so Pool
# time is serial with DVE time. Keep Pool idle; ACT has its own port
# and truly overlaps.
K_POOL = 0  # all gram tiles on DVE (f16)

_cached_nc = None


def _expected_graph():
    half = HALF
    offs = np.concatenate([np.arange(1, half + 1), N_ATOMS - np.arange(1, half + 1)])
    j = np.arange(N_ATOMS)
    nbr_dst = (j[:, None] + offs[None, :]) % N_ATOMS
    nbr_list = np.stack([np.repeat(j, DEG), nbr_dst.reshape(-1)], 1)
    o1, o2 = np.meshgrid(offs, offs, indexing="ij")
    keep = o1 != o2
    o1, o2 = o1[keep], o2[keep]
    i = (j[:, None] + o1[None, :]) % N_ATOMS
    k = (j[:, None] + o2[None, :]) % N_ATOMS
    jc = np.broadcast_to(j[:, None], i.shape)
    angle_list = np.stack([i.reshape(-1), jc.reshape(-1), k.reshape(-1)], 1)
    return nbr_list.astype(np.int64), angle_list.astype(np.int64)


def _graph_matches(nbr_list, angle_list):
    if nbr_list.shape != (N_ATOMS * DEG, 2):
        return False
    if angle_list.shape != (N_ATOMS * DEG * (DEG - 1), 3):
        return False
    exp_nbr, exp_ang = _expected_graph()
    return np.array_equal(np.asarray(nbr_list), exp_nbr) and np.array_equal(
        np.asarray(angle_list), exp_ang
    )


def _fallback_numpy(xyz, nbr_list, angle_list):
    """Exact numpy mirror of the jax reference (general graph)."""
    xyz = np.asarray(xyz, dtype=np.float32)
    nbr = np.asarray(nbr_list)
    ang = np.asarray(angle_list)
    E = nbr.shape[0]
    r_ji = xyz[ang[:, 0]] - xyz[ang[:, 1]]
    r_jk = xyz[ang[:, 2]] - xyz[ang[:, 1]]
    dot = np.sum(r_ji * r_jk, axis=-1)
    crs = np.linalg.norm(np.cross(r_ji, r_jk), axis=-1)
    alpha = np.arctan2(crs, dot)
    diff = xyz[nbr[:, 0]] - xyz[nbr[:, 1]]
    d = np.linalg.norm(diff, axis=-1)
    n = np.arange(1, N_RBF + 1, dtype=xyz.dtype)
    dc = (d / CUTOFF)[:, None]
    env = 1.0 / dc + A_ * dc ** (ENV_P - 1) + B_ * dc**ENV_P + C_ * dc ** (ENV_P + 1)
    e_rbf = env * np.sin(n * np.pi * dc)
    keys = nbr[:, 0] * N_ATOMS + nbr[:, 1]
    order = np.argsort(keys, kind="stable")
    ji_idx = order[np.searchsorted(keys[order], ang[:, 1] * N_ATOMS + ang[:, 0])]
    kj_idx = order[np.searchsorted(keys[order], ang[:, 2] * N_ATOMS + ang[:, 1])]
    trip = alpha[:, None] * e_rbf[kj_idx]
    out = np.zeros((E, N_RBF), dtype=np.float32)
    np.add.at(out, ji_idx, trip.astype(np.float32))
    return out


# ---------------------------------------------------------------------------
# Device kernel
# ---------------------------------------------------------------------------


def _build_device_kernel():
    import concourse.bacc as bacc
    import concourse.mybir as mybir
    from concourse.bass_types import AP
    from concourse.tile import TileContext

    F32 = mybir.dt.float32
    F16 = mybir.dt.float16
    I32 = mybir.dt.int32
    ALU = mybir.AluOpType
    ACT = mybir.ActivationFunctionType

    KP = K_POOL
    KD = TC - KP  # DVE gram tiles per chunk

    # Steer the activation-table-load pass: without this, Ln and Exp resolve
    # to different table sets and every Ln->Exp transition pays a 1.28us load.
    from concourse.hw_specs import get_activation_tables

    assign = {
        ACT.Ln: "natural_log_exp_and_others",
        ACT.Exp: "natural_log_exp_and_others",
        ACT.Sin: "trig_and_small",
        ACT.Arctan: "trig_and_small",
    }
    tabs = get_activation_tables("gen3")
    for name, fns in tabs.items():
        for fn, keep in assign.items():
            if name != keep:
                fns.discard(fn)

    nc = bacc.Bacc("TRN2", target_bir_lowering=False, debug=False, num_devices=N_CORES)
    win = nc.dram_tensor("win", [WIN_ROWS, 3], F32, kind="ExternalInput")
    consts = nc.dram_tensor("consts", [P, 8], F32, kind="ExternalInput")
    out = nc.dram_tensor("out", [J_PER_CORE * DEG, N_RBF], F32, kind="ExternalOutput")

    def sub(base: AP, off: int, dims) -> AP:
        """Sub-AP of an SBUF tile: keep partition dim, custom free dims."""
        return AP(
            tensor=base.tensor,
            offset=base.offset + off,
            ap=[list(base.ap[0]), *[list(d) for d in dims]],
        )

    with TileContext(nc) as tc:
        with (
            tc.tile_pool(name="big", bufs=1) as big,
            tc.tile_pool(name="work", bufs=2) as work,
        ):
            nco = big.tile([P, 8], F32, name="nco")
            nc.sync.dma_start(nco[:], consts[:])
            # register 0.5 (slot 0) for activation bias use
            nc.const_aps.aps[(F32, 0.5)] = sub(nco[:], 0, [[1, 1]])
            nc.const_aps.aps[(F32, 2.0)] = sub(nco[:], 1, [[1, 1]])

            # ---- persistent per-core buffers (free sizes per partition) ----
            w = big.tile([P, T * 51], F32, name="w")
            vc = big.tile([P, T * 48], F32, name="vc")  # V [t,c,b] f32
            n2a = big.tile([P, T * 16], F32, name="n2a")
            n2 = big.tile([P, T * 16], F32, name="n2")
            yn = big.tile([P, T * 16], F32, name="yn")  # 1/d
            xh = big.tile([P, T * 16], F32, name="xh")  # d/(2C)
            dcb = big.tile([P, T * 16], F32, name="dcb")  # d/C
            vh32 = big.tile([P, T * 48], F32, name="vh32")  # Vhat [t,c,b] f32
            vh16 = big.tile([P, T * 48], F16, name="vh16")  # Vhat [t,c,b] f16
            qq = big.tile([P, T * 16], F32, name="qq")
            x5 = big.tile([P, T * 16], F32, name="x5")
            hh = big.tile([P, T * 16], F32, name="hh")
            x4 = qq
            h2 = hh
            h3 = hh
            e5 = x5
            env2s = big.tile([P, T * 16], F16, name="env2s")  # 2*env/64 f16
            ki = big.tile([P, T * 16], I32, name="ki")
            kf = big.tile([P, T * 16], F32, name="kf")
            ff = big.tile([P, T * 16], F32, name="ff")  # frac(d/2C)
            uu = big.tile([P, T * 16], F32, name="uu")  # sin(pi f)
            u2 = big.tile([P, T * 16], F32, name="u2")
            tc1h = big.tile([P, T * 16], F16, name="tc1h")  # 2 cos(2pi f)
            s1f = big.tile([P, T * 16], F16, name="s1f")  # sin(2pi f)
            etm = big.tile([P, T * 16], F16, name="etm")  # recurrence tmp
            erbf = big.tile([P, T * 96], F16, name="erbf")  # [t,r,b] (= 2 env sin / 64)
            ch16 = big.tile([P, T * 256], F16, name="ch16")  # cos(a,b) [t,a,b]
            al2 = big.tile([P, T * 256], F16, name="al2")  # alpha/2 [t,a,b]

            def SR(buf, s, per, dims):
                return sub(buf[:], s * SCT * per, dims)

            def CR(buf, c, per, dims):
                return sub(buf[:], c * TC * per, dims)

            # =============== prep phases, per superchunk ===============
            for s in range(SC):
                # window DMA
                src = AP(
                    tensor=win,
                    offset=s * SCT * P * 3,
                    ap=[[3, P], [P * 3, SCT], [1, 51]],
                )
                nc.sync.dma_start(SR(w, s, 51, [[1, SCT * 51]]), src)

                # V build (DVE), c-major: V[t,c,b]; b 0..7 <- +1..+8, 8..15 <- -1..-8
                ctr = AP(
                    tensor=w.tensor,
                    offset=w[:].offset + s * SCT * 51 + 24,
                    ap=[list(w[:].ap[0]), [51, SCT], [1, 3], [0, 8]],
                )
                nc.vector.tensor_tensor(
                    SR(vc, s, 48, [[48, SCT], [16, 3], [1, 8]]),
                    AP(
                        tensor=w.tensor,
                        offset=w[:].offset + s * SCT * 51 + 27,
                        ap=[list(w[:].ap[0]), [51, SCT], [1, 3], [3, 8]],
                    ),
                    ctr,
                    ALU.subtract,
                )
                nc.vector.tensor_tensor(
                    sub(vc[:], s * SCT * 48 + 8, [[48, SCT], [16, 3], [1, 8]]),
                    AP(
                        tensor=w.tensor,
                        offset=w[:].offset + s * SCT * 51 + 21,
                        ap=[list(w[:].ap[0]), [51, SCT], [1, 3], [-3, 8]],
                    ),
                    ctr,
                    ALU.subtract,
                )

                # n2 = sum_c V^2: Square on ACT, two adds on DVE
                # vh32 doubles as V^2 scratch before being overwritten by Vhat
                nc.scalar.activation(
                    SR(vh32, s, 48, [[1, SCT * 48]]),
                    SR(vc, s, 48, [[1, SCT * 48]]),
                    ACT.Square,
                )
                nc.vector.tensor_tensor(
                    SR(n2a, s, 16, [[1, SCT * 16]]),
                    SR(vh32, s, 48, [[48, SCT], [1, 16]]),
                    sub(vh32[:], s * SCT * 48 + 16, [[48, SCT], [1, 16]]),
                    ALU.add,
                )
                nc.vector.tensor_tensor(
                    SR(n2, s, 16, [[1, SCT * 16]]),
                    SR(n2a, s, 16, [[1, SCT * 16]]),
                    sub(vh32[:], s * SCT * 48 + 32, [[48, SCT], [1, 16]]),
                    ALU.add,
                )

                # yn = 1/d = exp(-0.5 ln n2)  (ACT, natural_log_exp table)
                nc.scalar.activation(
                    SR(n2a, s, 16, [[1, SCT * 16]]), SR(n2, s, 16, [[1, SCT * 16]]),
                    ACT.Ln,
                )
                nc.scalar.activation(
                    SR(yn, s, 16, [[1, SCT * 16]]), SR(n2a, s, 16, [[1, SCT * 16]]),
                    ACT.Exp, scale=-0.5,
                )

                # x = d/(2C) ; dc = 2x ; Vhat = V * yn
                nc.vector.scalar_tensor_tensor(
                    SR(xh, s, 16, [[1, SCT * 16]]),
                    SR(n2, s, 16, [[1, SCT * 16]]),
                    1.0 / (2.0 * CUTOFF),
                    SR(yn, s, 16, [[1, SCT * 16]]),
                    ALU.mult, ALU.mult,
                )
                nc.scalar.activation(
                    SR(dcb, s, 16, [[1, SCT * 16]]),
                    SR(xh, s, 16, [[1, SCT * 16]]),
                    ACT.Copy, scale=2.0,
                )
                nc.vector.tensor_tensor(
                    SR(vh32, s, 48, [[48, SCT], [16, 3], [1, 16]]),
                    SR(vc, s, 48, [[48, SCT], [16, 3], [1, 16]]),
                    SR(yn, s, 16, [[16, SCT], [0, 3], [1, 16]]),
                    ALU.mult,
                )
                # vh16[t,b,c4] (lanes 0..2) <- cast of vh32[t,c,b]  (ACT Copy)
                nc.scalar.activation(
                    SR(vh16, s, 64, [[64, SCT], [4, 16], [1, 3]]),
                    SR(vh32, s, 48, [[48, SCT], [1, 16], [16, 3]]),
                    ACT.Copy,
                )

                # envelope: env2s = YN64*yn + x5*(EA64 + EB64*dc + EC64*dc^2)
                dcs = SR(dcb, s, 16, [[1, SCT * 16]])
                nc.scalar.activation(SR(qq, s, 16, [[1, SCT * 16]]), dcs, ACT.Square)
                nc.scalar.activation(
                    SR(x4, s, 16, [[1, SCT * 16]]),
                    SR(qq, s, 16, [[1, SCT * 16]]), ACT.Square,
                )
                nc.vector.tensor_tensor(
                    SR(x5, s, 16, [[1, SCT * 16]]),
                    SR(x4, s, 16, [[1, SCT * 16]]), dcs, ALU.mult,
                )
                nc.scalar.activation(
                    SR(hh, s, 16, [[1, SCT * 16]]), dcs, ACT.Copy,
                    scale=EC64, bias=EB64,
                )
                nc.vector.tensor_tensor(
                    SR(h2, s, 16, [[1, SCT * 16]]),
                    SR(hh, s, 16, [[1, SCT * 16]]), dcs, ALU.mult,
                )
                nc.scalar.activation(
                    SR(h3, s, 16, [[1, SCT * 16]]),
                    SR(h2, s, 16, [[1, SCT * 16]]), ACT.Copy, bias=EA64,
                )
                nc.vector.tensor_tensor(
                    SR(e5, s, 16, [[1, SCT * 16]]),
                    SR(x5, s, 16, [[1, SCT * 16]]),
                    SR(h3, s, 16, [[1, SCT * 16]]),
                    ALU.mult,
                )
                nc.vector.scalar_tensor_tensor(
                    SR(env2s, s, 16, [[1, SCT * 16]]),
                    SR(yn, s, 16, [[1, SCT * 16]]),
                    YN64,
                    SR(e5, s, 16, [[1, SCT * 16]]),
                    ALU.mult, ALU.add,
                )

                # f = frac(x) via int cast (x >= 0)
                nc.scalar.activation(
                    SR(ki, s, 16, [[1, SCT * 16]]), SR(xh, s, 16, [[1, SCT * 16]]),
                    ACT.Copy,
                )
                nc.scalar.activation(
                    SR(kf, s, 16, [[1, SCT * 16]]), SR(ki, s, 16, [[1, SCT * 16]]),
                    ACT.Copy,
                )
                nc.vector.tensor_tensor(
                    SR(ff, s, 16, [[1, SCT * 16]]),
                    SR(xh, s, 16, [[1, SCT * 16]]),
                    SR(kf, s, 16, [[1, SCT * 16]]),
                    ALU.subtract,
                )

                # u = sin(pi f) ; s1 = sin(2 pi f)  (trig table)
                nc.scalar.activation(
                    SR(uu, s, 16, [[1, SCT * 16]]), SR(ff, s, 16, [[1, SCT * 16]]),
                    ACT.Sin, scale=PI,
                )
                nc.scalar.activation(
                    SR(s1f, s, 16, [[1, SCT * 16]]), SR(ff, s, 16, [[1, SCT * 16]]),
                    ACT.Sin, scale=2.0 * PI,
                )

                # tc1 = 2 cos(2 pi f) = 2 - 4 u^2  (f16)
                nc.scalar.activation(
                    SR(u2, s, 16, [[1, SCT * 16]]),
                    SR(uu, s, 16, [[1, SCT * 16]]),
                    ACT.Square,
                )
                nc.scalar.activation(
                    SR(tc1h, s, 16, [[1, SCT * 16]]),
                    SR(u2, s, 16, [[1, SCT * 16]]),
                    ACT.Copy, scale=-4.0, bias=2.0,
                )

                # E recurrence -> erbf[t, r, b], r = n-1, all f16
                def EB(r, dims=None):
                    return sub(erbf[:], s * SCT * 96 + r * 16, [[96, SCT], [1, 16]])

                nc.vector.tensor_tensor(
                    EB(0), SR(env2s, s, 16, [[16, SCT], [1, 16]]),
                    SR(s1f, s, 16, [[16, SCT], [1, 16]]), ALU.mult,
                )
                tcs = SR(tc1h, s, 16, [[16, SCT], [1, 16]])
                nc.vector.tensor_tensor(EB(1), tcs, EB(0), ALU.mult)
                for r in range(2, 6):
                    nc.vector.tensor_tensor(
                        SR(etm, s, 16, [[16, SCT], [1, 16]]), tcs, EB(r - 1), ALU.mult
                    )
                    nc.vector.tensor_tensor(
                        EB(r), SR(etm, s, 16, [[16, SCT], [1, 16]]), EB(r - 2),
                        ALU.subtract,
                    )

            # ======== gram (per chunk) + alpha (per chunk-PAIR) + contraction ========
            for pair in range(CH // 2):
                for c in (2 * pair, 2 * pair + 1):
                    cb = c * TC
                    # ---- Gram on DVE, c-major: a-side replicated by one
                    # chunk-wide ACT copy (double-buffered so it never sits in
                    # the inter-chunk window); products + both c-plane sums 2x
                    repl = work.tile([P, KD * 768], F16, tag="repl", bufs=2)
                    p16 = work.tile([P, KD * 768], F16, tag="p16", bufs=1)
                    sum1 = work.tile([P, KD * 256], F16, tag="sum1", bufs=1)
                    nc.scalar.activation(
                        sub(repl[:], 0, [[256, 3 * KD], [16, 16], [1, 16]]),
                        sub(vh16[:], cb * 48, [[16, 3 * KD], [1, 16], [0, 16]]),
                        ACT.Copy,
                    )
                    for g in range(KD):
                        t = cb + KP + g
                        nc.vector.tensor_tensor(
                            sub(p16[:], g * 768, [[256, 3], [16, 16], [1, 16]]),
                            sub(repl[:], g * 768, [[256, 3], [16, 16], [1, 16]]),
                            sub(vh16[:], t * 48, [[16, 3], [0, 16], [1, 16]]),
                            ALU.mult,
                        )
                    nc.vector.tensor_tensor(
                        sub(sum1[:], 0, [[256, KD], [1, 256]]),
                        sub(p16[:], 0, [[768, KD], [1, 256]]),
                        sub(p16[:], 256, [[768, KD], [1, 256]]),
                        ALU.add,
                    )
                    nc.vector.tensor_tensor(
                        sub(ch16[:], (cb + KP) * 256, [[256, KD], [1, 256]]),
                        sub(sum1[:], 0, [[256, KD], [1, 256]]),
                        sub(p16[:], 512, [[768, KD], [1, 256]]),
                        ALU.add,
                    )

                pb = 2 * pair * TC * 256  # pair base offset in ch16/al2
                PN = 2 * TC * 256
                # ---- clamp to [-1, 1] (DVE, stays on the gram engine) ----
                nc.vector.tensor_scalar(
                    sub(ch16[:], pb, [[1, PN]]),
                    sub(ch16[:], pb, [[1, PN]]),
                    -1.0, 1.0, ALU.max, ALU.min,
                )

                # ---- alpha/2 = atan(exp(0.5 (ln((1-c)/2) - ln((1+c)/2)))) ----
                lnA = work.tile([P, PN], F16, tag="lnA", bufs=1)
                lnB = work.tile([P, PN], F16, tag="lnB", bufs=1)
                dln = lnB  # subtract writes in place; frees 8KB for repl
                nc.scalar.activation(
                    lnA[:], sub(ch16[:], pb, [[1, PN]]), ACT.Ln,
                    bias=0.5, scale=-0.5,
                )
                nc.scalar.activation(
                    lnB[:], sub(ch16[:], pb, [[1, PN]]), ACT.Ln,
                    bias=0.5, scale=0.5,
                )
                nc.vector.tensor_tensor(dln[:], lnA[:], lnB[:], ALU.subtract)
                tq = work.tile([P, PN], F16, tag="lnA", bufs=1)  # reuse lnA slot
                nc.scalar.activation(tq[:], dln[:], ACT.Exp, scale=0.5)
                nc.scalar.activation(
                    sub(al2[:], pb, [[1, PN]]), tq[:], ACT.Arctan
                )
                # diagonal alpha := 0 (i != k exclusion)
                dg = sub(al2[:], pb, [[256, 2 * TC], [17, 16]])
                nc.vector.tensor_scalar_mul(dg, dg, 0.0)

                # ---- contraction: ot[t,a,r] = sum_b al2[t,a,b] * erbf[t,r,b] ----
                for c in (2 * pair, 2 * pair + 1):
                    cb = c * TC
                p4 = work.tile([P, TC * 1536], F16, tag="p4", bufs=1)
                t1 = work.tile([P, TC * 768], F16, tag="t1", bufs=1)
                t2 = work.tile([P, TC * 384], F16, tag="t2", bufs=1)
                t3 = work.tile([P, TC * 192], F16, tag="t3", bufs=1)
                otc = work.tile([P, TC * 96], F32, tag="otc", bufs=2)
                for tl in range(TC):
                    t = cb + tl
                    nc.vector.tensor_tensor(
                        sub(p4[:], tl * 1536, [[96, 16], [16, 6], [1, 16]]),
                        sub(al2[:], t * 256, [[16, 16], [0, 6], [1, 16]]),
                        sub(erbf[:], t * 96, [[0, 16], [16, 6], [1, 16]]),
                        ALU.mult,
                    )
                nc.vector.tensor_tensor(
                    sub(t1[:], 0, [[768, TC], [8, 96], [1, 8]]),
                    sub(p4[:], 0, [[1536, TC], [16, 96], [1, 8]]),
                    sub(p4[:], 8, [[1536, TC], [16, 96], [1, 8]]),
                    ALU.add,
                )
                nc.vector.tensor_tensor(
                    sub(t2[:], 0, [[384, TC], [4, 96], [1, 4]]),
                    sub(t1[:], 0, [[768, TC], [8, 96], [1, 4]]),
                    sub(t1[:], 4, [[768, TC], [8, 96], [1, 4]]),
                    ALU.add,
                )
                nc.vector.tensor_tensor(
                    sub(t3[:], 0, [[192, TC], [2, 96], [1, 2]]),
                    sub(t2[:], 0, [[384, TC], [4, 96], [1, 2]]),
                    sub(t2[:], 2, [[384, TC], [4, 96], [1, 2]]),
                    ALU.add,
                )
                nc.vector.tensor_tensor(
                    sub(otc[:], 0, [[1, TC * 96]]),
                    sub(t3[:], 0, [[2, TC * 96]]),
                    sub(t3[:], 1, [[2, TC * 96]]),
                    ALU.add,
                )
                nc.vector.tensor_scalar(
                    sub(otc[:], 0, [[1, TC * 96]]),
                    sub(otc[:], 0, [[1, TC * 96]]),
                    OSCALE, None, ALU.mult,
                )
                # out rows (t*128 + p)*16 + a, cols r  (one DMA per chunk)
                dst = AP(
                    tensor=out,
                    offset=c * TC * P * 96,
                    ap=[[96, P], [96 * P, TC], [1, 96]],
                )
                nc.sync.dma_start(dst, sub(otc[:], 0, [[96, TC], [1, 96]]))

    nc.compile()
    return nc


def _get_nc():
    global _cached_nc
    if _cached_nc is None:
        _cached_nc = _build_device_kernel()
    return _cached_nc


def _make_consts():
    ncv = np.zeros(8, np.float32)
    ncv[0] = 0.5
    ncv[1] = 2.0
    return np.broadcast_to(ncv, (P, 8)).copy()


def _run_device(xyz, trace=False, tmpdir=None):
    from concourse import bass_utils

    nc = _get_nc()
    consts = _make_consts()
    ext = np.concatenate([xyz[-HALF:], xyz, xyz[:HALF]], axis=0)  # halo-extended
    in_maps = []
    for c in range(N_CORES):
        base = c * J_PER_CORE
        winc = np.ascontiguousarray(ext[base : base + WIN_ROWS])
        in_maps.append({"win": winc, "consts": consts})
    kwargs = {}
    if trace:
        kwargs = dict(trace=True)
        if tmpdir is not None:
            kwargs["tmpdir"] = tmpdir
    res = bass_utils.run_bass_kernel_spmd(
        nc, in_maps, core_ids=list(range(N_CORES)), **kwargs
    )
    shards = [res.results[c]["out"] for c in range(N_CORES)]
    full = np.concatenate(shards, axis=0).astype(np.float32)
    return full, res


def kernel(xyz, nbr_list, angle_list):
    xyz = np.asarray(xyz, dtype=np.float32)
    if not _graph_matches(nbr_list, angle_list):
        return _fallback_numpy(xyz, nbr_list, angle_list)
    out, _ = _run_device(xyz)
    return out
